# revision 1
# baseline (speedup 1.0000x reference)
"""Trainium2 Bass kernel for nn_Attention (dense_transformer).

Reference computation (per batch n of 4):
  qkv = W_qkv @ x + b          (384, 4096)   [x flattened to (256, 64*64)]
  raw C-order reinterpret of qkv flat buffer as (4096, 384) -> q|k|v (4096,128) each
  scores = q @ k.T / 64        (4096, 4096)
  soft = softmax(scores, axis=-2)             [column softmax]
  out = soft @ v               (4096, 128)
  raw reinterpret of out as (128, 4096)
  y = W_out @ out2 + b_out     (256, 4096)

Sharding: 8 cores = 4 batches x 2 column-chunks (j-axis of the score
matrix = rows of k/v). Column-softmax stats (over i) are local to a
j-chunk; each core produces a partial y, host sums the pair.

The SPMD graph is identical on all cores; the j-half selection is encoded
host-side by rotating the qkv output channels by 192 for odd cores (which
rotates the reinterpreted sequence axis by 2048) and rotating W_out's
e-axis by 64 to compensate on the output side.

Compute layout (per core):
  stage 1: F = W_qkv @ x + b as 3 o-tiles (128, 4096) bf16 -> DRAM fbuf,
           written as half-tiles fed by ACT(lo)/DVE(hi) bias-copies.
  loads:   qT (d,i) and kT (d,j) via xbar transpose-DMA from the (4096,384)
           reinterpret view of fbuf (one batched xbar window); v (j,d) plain
           via SWDGE. All split at 512-aligned boundaries per covering
           F o-tile so phase A starts before stage 1 fully drains.
  phase A (per j-block of 128): Pt[j,i] = exp(kT_jb.T q / 64), four
           (128,1024) exps with fused column-sum accum_out; Z -> 1/Z ->
           v scaled in place. The output matmuls for hw-groups 0-3
           accumulate inline in PSUM banks 4-7 (permuted i-axis
           i' = hb*128+e via a strided rhs AP on P, software-pipelined one
           j-block behind the stats), then drain through transpose/proj2.
  phase B+C+proj2 for groups 4-7, fused per 512-wide group: 16
           accumulate-MMs -> copy -> 4 TensorE transposes -> out2g ->
           proj2 MMs + bias -> y, per-half y DMAs; banks recycle via a
           bufs=2 pool. PSUM bank g == out2 group g throughout.
"""

import numpy as np
import ml_dtypes

import concourse.bass as bass
import concourse.bacc as bacc
import concourse.mybir as mybir
from concourse.bass_utils import run_bass_kernel_spmd
from concourse.tile import TileContext, add_dep_helper
from concourse.masks import make_identity

BF16 = mybir.dt.bfloat16
F32 = mybir.dt.float32
AF = mybir.ActivationFunctionType

N, C, E, O, HW = 4, 256, 128, 384, 4096
JC = HW // 2          # j-chunk per core
NJB = JC // 128       # 16 j-blocks
SCALE = 1.0 / 64.0    # 1/sqrt(HW)

_CACHE = {}


def build_nc():
    nc = bacc.Bacc("TRN2", target_bir_lowering=False, debug=False, num_devices=8)

    x_ext = nc.dram_tensor("x", [C, HW], BF16, kind="ExternalInput").ap()
    wqkvT_ext = nc.dram_tensor("wqkvT", [C, O], BF16, kind="ExternalInput").ap()
    bqkv_ext = nc.dram_tensor("bqkv", [O, 1], F32, kind="ExternalInput").ap()
    woutT_ext = nc.dram_tensor("woutT", [E, C], BF16, kind="ExternalInput").ap()
    bout_ext = nc.dram_tensor("bout", [C, 1], F32, kind="ExternalInput").ap()
    y_ext = nc.dram_tensor("out", [C, HW], BF16, kind="ExternalOutput").ap()

    fbuf = nc.dram_tensor("fbuf", [O * HW], BF16).ap()
    fview_o = fbuf.rearrange("(o hw) -> o hw", hw=HW)   # (384, 4096) write view
    fview_i = fbuf.rearrange("(i j) -> i j", j=O)        # (4096, 384) read view

    # persistent SBUF (fixed allocations; not subject to pool slot reuse).
    # qT/kT/v are split at 512-aligned boundaries covered by successive F
    # o-tiles so phase A can start before stage 1 fully drains.
    QSPL = [0, 1024, 2560, HW]       # parts covered by F o-tiles 0/1/2
    KSPL = [0, 1024, JC]             # parts covered by F o-tiles 0/1
    VSPL = [0, 1280, JC]
    qTp = [nc.alloc_sbuf_tensor(f"qT{i}", [128, QSPL[i + 1] - QSPL[i]], BF16).ap()
           for i in range(3)]
    kTp = [nc.alloc_sbuf_tensor(f"kT{i}", [128, KSPL[i + 1] - KSPL[i]], BF16).ap()
           for i in range(2)]
    vp = [nc.alloc_sbuf_tensor(f"v{i}", [128, VSPL[i + 1] - VSPL[i]], BF16).ap()
          for i in range(2)]

    def qT_sl(i0, w=512):
        p = 0 if i0 < 1024 else (1 if i0 < 2560 else 2)
        a = i0 - QSPL[p]
        assert a + w <= QSPL[p + 1] - QSPL[p]
        return qTp[p][:, a:a + w]

    def kT_sl(jb):
        p = 0 if jb < 8 else 1
        a = jb * 128 - KSPL[p]
        return kTp[p][:, a:a + 128]

    def v_sl(jb):
        p = 0 if jb < 10 else 1
        a = jb * 128 - VSPL[p]
        return vp[p][:, a:a + 128]

    zacc = nc.alloc_sbuf_tensor("zacc", [128, 64], F32).ap()
    zsum = nc.alloc_sbuf_tensor("zsum", [128, 16], F32).ap()
    zinv = nc.alloc_sbuf_tensor("zinv", [128, 16], F32).ap()
    outTg = [nc.alloc_sbuf_tensor(f"outTg{g}", [128, 512], BF16).ap()
             for g in range(8)]
    out2g = [nc.alloc_sbuf_tensor(f"out2g{g}", [128, 512], BF16).ap()
             for g in range(8)]
    P = nc.alloc_sbuf_tensor("P", [128, NJB * HW], BF16).ap()

    with TileContext(nc) as tc:
        with tc.tile_pool(name="consts", bufs=1) as consts:
            # ---- constants (bias first: it gates the first stage-1 copies) ----
            bias = consts.tile([128, 8], F32, name="bias", tag="bias")
            bq = [bias[:, i:i + 1] for i in range(3)]
            bo = [bias[:, 3 + i:4 + i] for i in range(2)]
            for ob in range(3):
                nc.scalar.dma_start(out=bq[ob], in_=bqkv_ext[ob * 128:(ob + 1) * 128, :])
            wq_all = consts.tile([128, 2 * O], BF16, name="wq_all", tag="wq_all")
            wqT = [wq_all[:, 0:O], wq_all[:, O:2 * O]]
            for cb in range(2):
                nc.scalar.dma_start(out=wqT[cb], in_=wqkvT_ext[cb * 128:(cb + 1) * 128, :])
            for cb in range(2):
                nc.scalar.dma_start(out=bo[cb], in_=bout_ext[cb * 128:(cb + 1) * 128, :])
            misc = consts.tile([128, C + 128], BF16, name="misc", tag="misc")
            woutT = misc[:, 0:C]
            ident = misc[:, C:C + 128]
            nc.scalar.dma_start(out=woutT, in_=woutT_ext[:])
            make_identity(nc, ident)
            scratch = consts.tile([128, 1], F32, name="scratch", tag="scratch")
            nc.vector.memset(scratch[:], 0.0)
            nc.scalar.activation(scratch[:], scratch[:], AF.Exp)

            # ---- PE warmup: dummy matmuls so HAM is at full clock before
            #      stage 1 (identity data; output never read) ----
            wsrc = consts.tile([128, 128], BF16, name="wsrc", tag="wsrc")
            nc.vector.memset(wsrc[:], 1.0)
            with tc.tile_pool(name="psW", bufs=1, space="PSUM") as psW:
                wtile = psW.tile([128, 128], F32, tag="warm")
                for _ in range(16):
                    nc.tensor.matmul(wtile[:], wsrc[:], wsrc[:], start=True, stop=True)

            # ---- x loads (2 x 1MB, sync ring) ----
            early = tc.alloc_tile_pool(name="early", bufs=1)
            # x split (cb, half) so the first matmuls start after 2 chunks
            xsb = [[early.tile([128, HW // 2], BF16, name=f"x{cb}{h}", tag=f"x{cb}{h}")
                    for h in range(2)] for cb in range(2)]
            Fsb = [[early.tile([128, HW // 2], BF16, name=f"F{i}{hh}", tag=f"F{i}{hh}")
                    for hh in range(2)] for i in range(3)]
            for h in range(2):
                for cb in range(2):
                    nc.sync.dma_start(
                        out=xsb[cb][h][:],
                        in_=x_ext[cb * 128:(cb + 1) * 128,
                                  h * (HW // 2):(h + 1) * (HW // 2)])

            # ---- stage 1: qkv projection -> Fsb o-tiles -> fbuf,
            #      with q/k/v part-loads woven in right after each F write ----
            with tc.tile_pool(name="psF", bufs=4, space="PSUM") as psF:
                f_writes = []
                vlds = []
                for ob in range(3):
                    for nch in range(8):
                        pf = psF.tile([128, 512], F32, tag="pf")
                        h, o512 = nch // 4, (nch % 4) * 512
                        sl = slice(nch * 512, (nch + 1) * 512)
                        nc.tensor.matmul(
                            pf[:], wqT[0][:, ob * 128:(ob + 1) * 128],
                            xsb[0][h][:, o512:o512 + 512],
                            start=True, stop=False,
                        )
                        nc.tensor.matmul(
                            pf[:], wqT[1][:, ob * 128:(ob + 1) * 128],
                            xsb[1][h][:, o512:o512 + 512],
                            start=False, stop=True,
                        )
                        fb = Fsb[ob][nch // 4]
                        fsl = slice((nch % 4) * 512, (nch % 4 + 1) * 512)
                        if nch < 4:
                            nc.vector.tensor_scalar_add(fb[:, fsl], pf[:], bq[ob])
                        else:
                            nc.scalar.activation(fb[:, fsl], pf[:], AF.Identity,
                                                 bias=bq[ob])
                    ws = []
                    for hh in range(2):
                        eng = nc.sync if hh == 0 else nc.scalar
                        ws.append(eng.dma_start(
                            out=fview_o[ob * 128:(ob + 1) * 128,
                                        hh * (HW // 2):(hh + 1) * (HW // 2)],
                            in_=Fsb[ob][hh][:],
                        ))
                    f_writes.append(ws)
                    # v loads (plain copies, scalar ring) right after their F write
                    if ob == 0:
                        r = nc.gpsimd.dma_start(
                            out=vp[0].rearrange("p (t d) -> p t d", d=128),
                            in_=fview_i[0:1280, 2 * E:3 * E].rearrange(
                                "(t p) d -> p t d", p=128))
                        vlds.append(r)
                        for w2 in ws:
                            add_dep_helper(r.ins, w2.ins, reason="fbuf RAW")
                    elif ob == 1:
                        r = nc.gpsimd.dma_start(
                            out=vp[1].rearrange("p (t d) -> p t d", d=128),
                            in_=fview_i[1280:2048, 2 * E:3 * E].rearrange(
                                "(t p) d -> p t d", p=128))
                        vlds.append(r)
                        for w2 in ws:
                            add_dep_helper(r.ins, w2.ins, reason="fbuf RAW")
                # all xbar transposes batched in one mode-window (sync ring).
                # The first three only order behind F0/F1 so they run before
                # F2's writes; v loads are pushed behind the transposes in the
                # bandwidth queue (v is not needed until the first stats).
                tr_specs = [
                    (qTp[0], fview_i[0:1024, 0:E], 0, 2),
                    (kTp[0], fview_i[0:1024, E:2 * E], 0, 2),
                    (qTp[1], fview_i[1024:2560, 0:E], 1, 2),
                    (qTp[2], fview_i[2560:HW, 0:E], 2, 3),
                    (kTp[1], fview_i[1024:2048, E:2 * E], 1, 3),
                ]
                trs = []
                for dst, srcap, dep, nhint in tr_specs:
                    rt = nc.sync.dma_start_transpose(out=dst[:], in_=srcap)
                    trs.append(rt)
                    for w2 in f_writes[dep]:
                        add_dep_helper(rt.ins, w2.ins, reason="fbuf RAW")
                    for ws2 in f_writes[:nhint]:
                        for w2 in ws2:
                            add_dep_helper(rt.ins, w2.ins, sync=False,
                                           reason="xbar window after copies")
            early.release()

            # ---- phase A: scores + exp(1024-wide, fused column sums),
            #      with groups 0-3 of the output matmul inlined (banks 4-7),
            #      software-pipelined one j-block behind the stats ----
            P3 = P.rearrange("p (jb e hb) -> p jb hb e", jb=NJB, hb=32)
            with tc.tile_pool(name="psBi", bufs=1, space="PSUM") as psBi:
                obi = [psBi.tile([128, 512], F32, name=f"obi{g}", tag=f"obi{g}")
                       for g in range(4)]

                def inline_mms(jb):
                    for g in range(4):
                        nc.tensor.matmul(
                            obi[g][:], v_sl(jb), P3[:, jb, 4 * g:4 * g + 4, :],
                            start=(jb == 0), stop=(jb == NJB - 1),
                        )

                with tc.tile_pool(name="psA", bufs=2, space="PSUM") as psA:
                    def score_exp(jb, h):
                        pa = psA.tile([128, 1024], F32, tag="pa")
                        for n2 in range(2):
                            i0 = h * 1024 + n2 * 512
                            nc.tensor.matmul(
                                pa[:, n2 * 512:(n2 + 1) * 512],
                                kT_sl(jb), qT_sl(i0),
                                start=True, stop=True,
                            )
                        nc.scalar.activation(
                            out=P[:, jb * HW + h * 1024: jb * HW + (h + 1) * 1024],
                            in_=pa[:],
                            func=AF.Exp,
                            scale=SCALE,
                            accum_out=zacc[:, jb * 4 + h: jb * 4 + h + 1],
                        )

                    # h0/h1 only touch qT parts 0-1; h2/h3 need part 2 which
                    # lands last. Lead with h0/h1 of the first three j-blocks
                    # so the exp chain stays dense while qT2 is in flight.
                    LEAD = 4
                    for jb in range(LEAD):
                        score_exp(jb, 0)
                    for jb in range(LEAD):
                        score_exp(jb, 1)
                    for jb in range(NJB):
                        score_exp(jb, 2)
                        score_exp(jb, 3)
                        nc.vector.reduce_sum(
                            out=zsum[:, jb:jb + 1],
                            in_=zacc[:, jb * 4:(jb + 1) * 4],
                            axis=mybir.AxisListType.X,
                        )
                        nc.vector.reciprocal(zinv[:, jb:jb + 1], zsum[:, jb:jb + 1])
                        nc.vector.tensor_scalar_mul(
                            v_sl(jb), v_sl(jb), zinv[:, jb:jb + 1],
                        )
                        if jb + LEAD < NJB:
                            score_exp(jb + LEAD, 0)
                            score_exp(jb + LEAD, 1)
                        if jb > 2:
                            inline_mms(jb - 3)
                    inline_mms(NJB - 3)
                    inline_mms(NJB - 2)
                    inline_mms(NJB - 1)

                # drain inline groups 0-3 through transpose/proj2 (C-part only)
                with tc.tile_pool(name="psC0", bufs=2, space="PSUM") as psC0, \
                     tc.tile_pool(name="psY0", bufs=2, space="PSUM") as psY0, \
                     tc.tile_pool(name="late0", bufs=1) as late0:
                    yg0 = [[late0.tile([128, 512], BF16, name=f"yg{cb}_{g}",
                                       tag=f"yg{cb}_{g}") for g in range(4)]
                           for cb in range(2)]
                    for g in range(4):
                        if g % 2 == 0:
                            nc.scalar.copy(outTg[g][:], obi[g][:])
                        else:
                            nc.vector.tensor_copy(outTg[g][:], obi[g][:])
                        tp = psC0.tile([128, 512], BF16, tag="tp0")
                        for s in range(4):
                            nc.tensor.transpose(
                                tp[:, s * 128:(s + 1) * 128],
                                outTg[g][:, s * 128:(s + 1) * 128],
                                ident,
                            )
                        if g % 2 == 0:
                            nc.vector.tensor_copy(out2g[g][:], tp[:])
                        else:
                            nc.scalar.copy(out2g[g][:], tp[:])
                        for cb in range(2):
                            py = psY0.tile([128, 512], F32, tag="py0")
                            nc.tensor.matmul(
                                py[:], woutT[:, cb * 128:(cb + 1) * 128], out2g[g][:],
                                start=True, stop=True,
                            )
                            dst = yg0[cb][g][:]
                            if cb == 0:
                                nc.scalar.activation(dst, py[:], AF.Identity,
                                                     bias=bo[cb])
                            else:
                                nc.vector.tensor_scalar_add(dst, py[:], bo[cb])
                            [nc.sync, nc.scalar][cb].dma_start(
                                out=y_ext[cb * 128:(cb + 1) * 128,
                                          g * 512:(g + 1) * 512],
                                in_=dst)

            # ---- phase B + C + proj2, fused per 512-wide group ----
            # outT is produced with permuted i-axis: i' = hb*128 + e (hb = hw
            # block, e = embed row), so PSUM bank g holds exactly the data for
            # out2 group g: transpose outT'[:, hb*128:+128].T = out2[:, hb*128:+128].
            # The permutation comes free via a strided rhs AP on P.
            with tc.tile_pool(name="psB", bufs=2, space="PSUM") as psB, \
                 tc.tile_pool(name="psC", bufs=2, space="PSUM") as psC, \
                 tc.tile_pool(name="psY", bufs=3, space="PSUM") as psY, \
                 tc.tile_pool(name="late", bufs=1) as late:
                yg1 = [[late.tile([128, 512], BF16, name=f"yb{cb}_{g}",
                                  tag=f"yb{cb}_{g}") for g in range(4)]
                       for cb in range(2)]
                for g in range(4, 8):
                    ob_ps = psB.tile([128, 512], F32, tag="ob_ps")
                    for jb in range(NJB):
                        nc.tensor.matmul(
                            ob_ps[:],
                            v_sl(jb),
                            P3[:, jb, 4 * g:4 * g + 4, :],
                            start=(jb == 0), stop=(jb == NJB - 1),
                        )
                    if g % 2 == 0:
                        nc.scalar.copy(outTg[g][:], ob_ps[:])
                    else:
                        nc.vector.tensor_copy(outTg[g][:], ob_ps[:])
                    tp = psC.tile([128, 512], BF16, tag="tp")
                    for s in range(4):
                        nc.tensor.transpose(
                            tp[:, s * 128:(s + 1) * 128],
                            outTg[g][:, s * 128:(s + 1) * 128],
                            ident,
                        )
                    if g % 2 == 0:
                        nc.vector.tensor_copy(out2g[g][:], tp[:])
                    else:
                        nc.scalar.copy(out2g[g][:], tp[:])
                    for cb in range(2):
                        py = psY.tile([128, 512], F32, tag="py")
                        nc.tensor.matmul(
                            py[:], woutT[:, cb * 128:(cb + 1) * 128], out2g[g][:],
                            start=True, stop=True,
                        )
                        dst = yg1[cb][g - 4][:]
                        if cb == 0:
                            nc.scalar.activation(dst, py[:], AF.Identity, bias=bo[cb])
                        else:
                            nc.vector.tensor_scalar_add(dst, py[:], bo[cb])
                        [nc.sync, nc.scalar][cb].dma_start(
                            out=y_ext[cb * 128:(cb + 1) * 128,
                                      g * 512:(g + 1) * 512],
                            in_=dst)

    nc.compile()
    return nc


def get_nc():
    if "nc" not in _CACHE:
        _CACHE["nc"] = build_nc()
    return _CACHE["nc"]


def make_in_maps(x, W_qkv, b_qkv, W_out, b_out):
    x = np.asarray(x, dtype=np.float32)
    W_qkv = np.asarray(W_qkv, dtype=np.float32)
    b_qkv = np.asarray(b_qkv, dtype=np.float32)
    W_out = np.asarray(W_out, dtype=np.float32)
    b_out = np.asarray(b_out, dtype=np.float32)

    operm = (np.arange(O) + O // 2) % O      # rotate qkv channels by 192
    eperm = (np.arange(E) + E // 2) % E      # rotate e-axis by 64

    halves = []
    for h in range(2):
        if h == 0:
            wq, bqv, wo, bov = W_qkv, b_qkv, W_out, b_out
        else:
            wq = W_qkv[operm]
            bqv = b_qkv[operm]
            wo = W_out[:, eperm]
            bov = np.zeros_like(b_out)
        halves.append({
            "wqkvT": np.ascontiguousarray(wq.T).astype(ml_dtypes.bfloat16),
            "bqkv": np.ascontiguousarray(bqv.reshape(O, 1)),
            "woutT": np.ascontiguousarray(wo.T).astype(ml_dtypes.bfloat16),
            "bout": np.ascontiguousarray(bov.reshape(C, 1)),
        })

    xb = [np.ascontiguousarray(x[n].reshape(C, HW)).astype(ml_dtypes.bfloat16)
          for n in range(N)]
    in_maps = []
    for core in range(8):
        n, h = core // 2, core % 2
        m = {"x": xb[n]}
        m.update(halves[h])
        in_maps.append(m)
    return in_maps


def run(inputs, trace=False, **kw):
    nc = get_nc()
    in_maps = make_in_maps(**inputs)
    res = run_bass_kernel_spmd(nc, in_maps, core_ids=list(range(8)), trace=trace, **kw)
    ys = [np.asarray(res.results[i]["out"], dtype=np.float32) for i in range(8)]
    y = np.stack([ys[2 * n] + ys[2 * n + 1] for n in range(N)])
    return y.reshape(N, C, 64, 64), res


def kernel(**inputs):
    y, _ = run(inputs, trace=False)
    return y



# revision 2
# speedup vs baseline: 1.0249x; 1.0249x over previous
"""Trainium2 Bass kernel for nn_Attention (dense_transformer).

Reference computation (per batch n of 4):
  qkv = W_qkv @ x + b          (384, 4096)   [x flattened to (256, 64*64)]
  raw C-order reinterpret of qkv flat buffer as (4096, 384) -> q|k|v (4096,128) each
  scores = q @ k.T / 64        (4096, 4096)
  soft = softmax(scores, axis=-2)             [column softmax]
  out = soft @ v               (4096, 128)
  raw reinterpret of out as (128, 4096)
  y = W_out @ out2 + b_out     (256, 4096)

Sharding: 8 cores = 4 batches x 2 column-chunks (j-axis of the score
matrix = rows of k/v). Column-softmax stats (over i) are local to a
j-chunk; each core produces a partial y, host sums the pair.  The SPMD
graph is identical on all cores; the j-half selection is encoded
host-side by rotating the qkv output channels by 192 for odd cores and
rotating W_out's e-axis by 64 to compensate.

Compute layout (per core), v2 (ACT-bound redesign):
  stage 1: F = W_qkv @ x + b as 3 o-tiles (128, 4096) bf16 -> DRAM fbuf
           (x loads split across sync+scalar rings; drains ACT/DVE).
  loads:   qT (d,i), kT (d,j) via xbar transpose-DMA; v (j,d) via SWDGE.
  phase A: per (jb, half) chunk: 4 score MMs bf16 -> (128,2048) PSUM
           (2 bufs x 4 banks = all 8), one ACT exp with accum_out.
           ACT does NOTHING else in phase A (32 x ~2.08us = the
           critical path).  Sweep order jb0-7xh0, jb0-7xh1(+stats),
           jb8-15xh0, jb8-15xh1(+stats) so exps start once F0/F1 land.
  stats:   DVE: zsum = reduce(zacc pair); szinv = 1/(zsum/4096) = 4096/Z.
  converts (DVE/Pool, hidden under ACT): dPn8 = fp8e4(P*szinv - 1),
           scattered into P8 so that P8 column i' = hb*128+e2 holds
           score row i = e2*32+hb (proj2-transpose-friendly order).
           v8 = fp8e4(v) plain convert (no zinv on v).
  tail:    colsum[e] = sum_j v_bf16[j,e] (16 trivial MMs, exact-ish
           rank-1 term of P/Z*4096 = 1 + dPn); per 512-wide group g:
           8 fp8 DoubleRow pair-MMs (4x PE) accumulate dPn8 @ v8,
           drain with bias=colsum, 4 TensorE transposes, out2 copy,
           proj2 MMs (woutT pre-scaled by 1/4096 host-side) + b_out,
           y DMAs on sync/scalar rings.
"""

import numpy as np
import ml_dtypes

import concourse.bass as bass
import concourse.bacc as bacc
import concourse.mybir as mybir
from concourse.bass_utils import run_bass_kernel_spmd
from concourse.tile import TileContext, add_dep_helper
from concourse.masks import make_identity

BF16 = mybir.dt.bfloat16
F32 = mybir.dt.float32
FP8 = mybir.dt.float8e4
AF = mybir.ActivationFunctionType
ALU = mybir.AluOpType
DR = mybir.MatmulPerfMode.DoubleRow

N, C, E, O, HW = 4, 256, 128, 384, 4096
JC = HW // 2          # j-chunk per core
NJB = JC // 128       # 16 j-blocks
SCALE = 1.0 / 64.0    # 1/sqrt(HW)
SFIX = 4096.0         # softmax renorm: Pn = P * (4096/Z), undone in W_out

_CACHE = {}


def build_nc():
    nc = bacc.Bacc("TRN2", target_bir_lowering=False, debug=False, num_devices=8)

    x_ext = nc.dram_tensor("x", [C, HW], BF16, kind="ExternalInput").ap()
    wqkvT_ext = nc.dram_tensor("wqkvT", [C, O], BF16, kind="ExternalInput").ap()
    bqkv_ext = nc.dram_tensor("bqkv", [O, 1], F32, kind="ExternalInput").ap()
    woutT_ext = nc.dram_tensor("woutT", [E, C], BF16, kind="ExternalInput").ap()
    bout_ext = nc.dram_tensor("bout", [C, 1], F32, kind="ExternalInput").ap()
    y_ext = nc.dram_tensor("out", [C, HW], BF16, kind="ExternalOutput").ap()

    fbuf = nc.dram_tensor("fbuf", [O * HW], BF16).ap()
    fview_o = fbuf.rearrange("(o hw) -> o hw", hw=HW)   # (384, 4096) write view
    fview_i = fbuf.rearrange("(i j) -> i j", j=O)        # (4096, 384) read view

    # persistent SBUF.  qT/kT/v split at 512-aligned boundaries covered by
    # successive F o-tiles so phase A starts before stage 1 fully drains.
    QSPL = [0, 1024, 2560, HW]       # parts covered by F o-tiles 0/1/2
    KSPL = [0, 1024, JC]             # parts covered by F o-tiles 0/1
    VSPL = [0, 1280, JC]
    qTp = [nc.alloc_sbuf_tensor(f"qT{i}", [128, QSPL[i + 1] - QSPL[i]], BF16).ap()
           for i in range(3)]
    kTp = [nc.alloc_sbuf_tensor(f"kT{i}", [128, KSPL[i + 1] - KSPL[i]], BF16).ap()
           for i in range(2)]
    vp = [nc.alloc_sbuf_tensor(f"v{i}", [128, VSPL[i + 1] - VSPL[i]], BF16).ap()
          for i in range(2)]

    def qT_sl(i0, w=512):
        p = 0 if i0 < 1024 else (1 if i0 < 2560 else 2)
        a = i0 - QSPL[p]
        assert a + w <= QSPL[p + 1] - QSPL[p]
        return qTp[p][:, a:a + w]

    def kT_sl(jb):
        p = 0 if jb < 8 else 1
        a = jb * 128 - KSPL[p]
        return kTp[p][:, a:a + 128]

    def v_sl(jb):
        p = 0 if jb < 10 else 1
        a = jb * 128 - VSPL[p]
        return vp[p][:, a:a + 128]

    v8 = nc.alloc_sbuf_tensor("v8", [128, JC], FP8).ap()       # (j, e) fp8
    zacc = nc.alloc_sbuf_tensor("zacc", [128, 32], F32).ap()   # per (jb, half)
    zsum = nc.alloc_sbuf_tensor("zsum", [128, 16], F32).ap()
    ztmp = nc.alloc_sbuf_tensor("ztmp", [128, 16], F32).ap()
    szinv = nc.alloc_sbuf_tensor("szinv", [128, 16], F32).ap()  # 4096/Z per jb
    cs_sb = nc.alloc_sbuf_tensor("cs_sb", [128, 1], F32).ap()   # colsum(v)
    outTg = [nc.alloc_sbuf_tensor(f"outTg{g}", [128, 512], BF16).ap()
             for g in range(8)]
    out2g = [nc.alloc_sbuf_tensor(f"out2g{g}", [128, 512], BF16).ap()
             for g in range(8)]
    # dPn8, stored so that per jb, column i' = hb*128 + e2 holds data for
    # score row i = e2*32 + hb (the proj2-transpose order).
    P8 = nc.alloc_sbuf_tensor("P8", [128, NJB * HW], FP8).ap()
    # bf16 exp ring (plain i-order), converted+scattered into P8.
    PR = 4
    Pring = nc.alloc_sbuf_tensor("Pring", [128, PR * 2048], BF16).ap()

    def pring_sl(ci):
        s = (ci % PR) * 2048
        return Pring[:, s:s + 2048]

    def p8_dst(jb, half):
        # out AP dims (e2: 64 @1 offset 64*half, hb: 32 @128) within jb region
        reg = P8[:, jb * HW:(jb + 1) * HW]
        v = reg.rearrange("p (hb e) -> p e hb", hb=32)
        return v[:, 64 * half:64 * (half + 1), :]

    def p8_rhs(t, g):
        # rhs for (pair t, group g): dims (p, jb-pair 2 @4096, 512 @1)
        reg = P8[:, t * 2 * HW:(t + 1) * 2 * HW]
        v = reg.rearrange("p (two x) -> p two x", two=2)
        return v[:, :, g * 512:(g + 1) * 512]

    with TileContext(nc) as tc:
        with tc.tile_pool(name="consts", bufs=1) as consts:
            # ---- constants (bias first: it gates the first stage-1 copies) ----
            bias = consts.tile([128, 8], F32, name="bias", tag="bias")
            bq = [bias[:, i:i + 1] for i in range(3)]
            bo = [bias[:, 3 + i:4 + i] for i in range(2)]
            for ob in range(3):
                nc.scalar.dma_start(out=bq[ob], in_=bqkv_ext[ob * 128:(ob + 1) * 128, :])
            wq_all = consts.tile([128, 2 * O], BF16, name="wq_all", tag="wq_all")
            wqT = [wq_all[:, 0:O], wq_all[:, O:2 * O]]
            for cb in range(2):
                nc.scalar.dma_start(out=wqT[cb], in_=wqkvT_ext[cb * 128:(cb + 1) * 128, :])
            for cb in range(2):
                nc.scalar.dma_start(out=bo[cb], in_=bout_ext[cb * 128:(cb + 1) * 128, :])
            misc = consts.tile([128, C + 128 + 8], BF16, name="misc", tag="misc")
            woutT = misc[:, 0:C]
            ident = misc[:, C:C + 128]
            ones16 = misc[:, C + 128:C + 129]
            nc.scalar.dma_start(out=woutT, in_=woutT_ext[:])
            make_identity(nc, ident)
            nc.vector.memset(ones16, 1.0)
            scratch = consts.tile([128, 1], F32, name="scratch", tag="scratch")
            nc.vector.memset(scratch[:], 0.0)
            nc.scalar.activation(scratch[:], scratch[:], AF.Exp)

            # ---- PE warmup: dummy matmuls so HAM is at full clock ----
            wsrc = consts.tile([128, 128], BF16, name="wsrc", tag="wsrc")
            nc.vector.memset(wsrc[:], 1.0)
            with tc.tile_pool(name="psW", bufs=1, space="PSUM") as psW:
                wtile = psW.tile([128, 128], F32, tag="warm")
                for _ in range(16):
                    nc.tensor.matmul(wtile[:], wsrc[:], wsrc[:], start=True, stop=True)

            # ---- x loads (2 x 1MB), split across sync+scalar rings ----
            early = tc.alloc_tile_pool(name="early", bufs=1)
            xsb = [[early.tile([128, HW // 2], BF16, name=f"x{cb}{h}", tag=f"x{cb}{h}")
                    for h in range(2)] for cb in range(2)]
            Fsb = [[early.tile([128, HW // 2], BF16, name=f"F{i}{hh}", tag=f"F{i}{hh}")
                    for hh in range(2)] for i in range(3)]
            for h in range(2):
                for cb in range(2):
                    eng = nc.sync if h == 0 else nc.scalar
                    eng.dma_start(
                        out=xsb[cb][h][:],
                        in_=x_ext[cb * 128:(cb + 1) * 128,
                                  h * (HW // 2):(h + 1) * (HW // 2)])

            # ---- stage 1: qkv projection -> Fsb o-tiles -> fbuf,
            #      with q/k/v part-loads woven in right after each F write ----
            with tc.tile_pool(name="psF", bufs=4, space="PSUM") as psF:
                f_writes = []
                vlds = []
                for ob in range(3):
                    for nch in range(8):
                        pf = psF.tile([128, 512], F32, tag="pf")
                        h, o512 = nch // 4, (nch % 4) * 512
                        nc.tensor.matmul(
                            pf[:], wqT[0][:, ob * 128:(ob + 1) * 128],
                            xsb[0][h][:, o512:o512 + 512],
                            start=True, stop=False,
                        )
                        nc.tensor.matmul(
                            pf[:], wqT[1][:, ob * 128:(ob + 1) * 128],
                            xsb[1][h][:, o512:o512 + 512],
                            start=False, stop=True,
                        )
                        fb = Fsb[ob][nch // 4]
                        fsl = slice((nch % 4) * 512, (nch % 4 + 1) * 512)
                        if nch < 4:
                            nc.vector.tensor_scalar_add(fb[:, fsl], pf[:], bq[ob])
                        else:
                            nc.scalar.activation(fb[:, fsl], pf[:], AF.Identity,
                                                 bias=bq[ob])
                    ws = []
                    for hh in range(2):
                        eng = nc.sync if hh == 0 else nc.scalar
                        ws.append(eng.dma_start(
                            out=fview_o[ob * 128:(ob + 1) * 128,
                                        hh * (HW // 2):(hh + 1) * (HW // 2)],
                            in_=Fsb[ob][hh][:],
                        ))
                    f_writes.append(ws)
                    # v loads (plain copies, SWDGE) right after their F write
                    if ob == 0:
                        r = nc.gpsimd.dma_start(
                            out=vp[0].rearrange("p (t d) -> p t d", d=128),
                            in_=fview_i[0:1280, 2 * E:3 * E].rearrange(
                                "(t p) d -> p t d", p=128))
                        vlds.append(r)
                        for w2 in ws:
                            add_dep_helper(r.ins, w2.ins, reason="fbuf RAW")
                    elif ob == 1:
                        r = nc.gpsimd.dma_start(
                            out=vp[1].rearrange("p (t d) -> p t d", d=128),
                            in_=fview_i[1280:2048, 2 * E:3 * E].rearrange(
                                "(t p) d -> p t d", p=128))
                        vlds.append(r)
                        for w2 in ws:
                            add_dep_helper(r.ins, w2.ins, reason="fbuf RAW")
                # all xbar transposes batched in one mode-window (sync ring).
                tr_specs = [
                    (qTp[0], fview_i[0:1024, 0:E], 0, 2),
                    (kTp[0], fview_i[0:1024, E:2 * E], 0, 2),
                    (qTp[1], fview_i[1024:2560, 0:E], 1, 2),
                    (qTp[2], fview_i[2560:HW, 0:E], 2, 3),
                    (kTp[1], fview_i[1024:2048, E:2 * E], 1, 3),
                ]
                for dst, srcap, dep, nhint in tr_specs:
                    rt = nc.sync.dma_start_transpose(out=dst[:], in_=srcap)
                    for w2 in f_writes[dep]:
                        add_dep_helper(rt.ins, w2.ins, reason="fbuf RAW")
                    for ws2 in f_writes[:nhint]:
                        for w2 in ws2:
                            add_dep_helper(rt.ins, w2.ins, sync=False,
                                           reason="xbar window after copies")
            early.release()

            # v8 = fp8(v) plain converts (DVE), as soon as v parts land
            nc.vector.tensor_copy(v8[:, 0:1280], vp[0][:])
            nc.vector.tensor_copy(v8[:, 1280:JC], vp[1][:])

            # ---- phase A: scores + exp(2048-wide, fused column accum).
            #      ACT does only exps; DVE stats; DVE/Pool dPn8 converts. ----
            with tc.tile_pool(name="psA", bufs=2, space="PSUM") as psA:
                def score_exp(jb, half):
                    ci = jb * 2 + half
                    pa = psA.tile([128, 2048], F32, tag="pa")
                    for n2 in range(4):
                        i0 = half * 2048 + n2 * 512
                        nc.tensor.matmul(
                            pa[:, n2 * 512:(n2 + 1) * 512],
                            kT_sl(jb), qT_sl(i0),
                            start=True, stop=True,
                        )
                    nc.scalar.activation(
                        out=pring_sl(ci),
                        in_=pa[:],
                        func=AF.Exp,
                        scale=SCALE,
                        accum_out=zacc[:, ci:ci + 1],
                    )

                def stats(jb):
                    nc.vector.reduce_sum(
                        out=zsum[:, jb:jb + 1],
                        in_=zacc[:, 2 * jb:2 * jb + 2],
                        axis=mybir.AxisListType.X,
                    )
                    nc.vector.tensor_scalar_mul(
                        ztmp[:, jb:jb + 1], zsum[:, jb:jb + 1], 1.0 / SFIX)
                    nc.vector.reciprocal(szinv[:, jb:jb + 1], ztmp[:, jb:jb + 1])

                def convert(jb, half, eng):
                    ci = jb * 2 + half
                    src = pring_sl(ci).rearrange("p (e hb) -> p e hb", hb=32)
                    eng.tensor_scalar(
                        out=p8_dst(jb, half),
                        in0=src,
                        scalar1=szinv[:, jb:jb + 1],
                        scalar2=1.0,
                        op0=ALU.mult,
                        op1=ALU.subtract,
                    )

                # sweep 1: (jb 0-7) x h0 — needs only qT parts 0-1, kT part 0
                for jb in range(8):
                    score_exp(jb, 0)
                # sweep 2: (jb 0-7) x h1 + stats + converts
                for jb in range(8):
                    score_exp(jb, 1)
                    stats(jb)
                    convert(jb, 0, nc.vector)
                    convert(jb, 1, nc.gpsimd)
                # sweep 3: (jb 8-15) x h0
                for jb in range(8, NJB):
                    score_exp(jb, 0)
                # sweep 4: (jb 8-15) x h1 + stats + converts
                for jb in range(8, NJB):
                    score_exp(jb, 1)
                    stats(jb)
                    convert(jb, 0, nc.gpsimd)
                    convert(jb, 1, nc.vector)

            # ---- tail: colsum, fp8 DoubleRow out-MMs, transpose, proj2 ----
            with tc.tile_pool(name="psCS", bufs=1, space="PSUM") as psCS, \
                 tc.tile_pool(name="psB", bufs=2, space="PSUM") as psB, \
                 tc.tile_pool(name="psC", bufs=2, space="PSUM") as psC, \
                 tc.tile_pool(name="psY", bufs=2, space="PSUM") as psY, \
                 tc.tile_pool(name="late", bufs=1) as late:
                # colsum[e] = sum_j v[j, e] in bf16 (rank-1 term of Pn)
                cs_ps = psCS.tile([128, 1], F32, tag="cs")
                for jb in range(NJB):
                    nc.tensor.matmul(
                        cs_ps[:], v_sl(jb), ones16,
                        start=(jb == 0), stop=(jb == NJB - 1),
                    )
                nc.vector.tensor_copy(cs_sb[:], cs_ps[:])

                yg = [[late.tile([128, 512], BF16, name=f"yb{cb}_{g}",
                                 tag=f"yb{cb}_{g}") for g in range(8)]
                      for cb in range(2)]
                for g in range(8):
                    ob_ps = psB.tile([128, 512], F32, tag="ob_ps")
                    for t in range(NJB // 2):
                        v8pair = v8[:, t * 256:(t + 1) * 256].rearrange(
                            "p (two e) -> p two e", two=2)
                        nc.tensor.matmul(
                            ob_ps[:], v8pair, p8_rhs(t, g),
                            start=(t == 0), stop=(t == NJB // 2 - 1),
                            perf_mode=DR,
                        )
                    # drain with colsum bias; rotate engines (ACT idle in tail)
                    if g % 2 == 0:
                        nc.scalar.activation(outTg[g][:], ob_ps[:], AF.Identity,
                                             bias=cs_sb[:])
                    else:
                        nc.vector.tensor_scalar_add(outTg[g][:], ob_ps[:], cs_sb[:])
                    tp = psC.tile([128, 512], BF16, tag="tp")
                    for s in range(4):
                        nc.tensor.transpose(
                            tp[:, s * 128:(s + 1) * 128],
                            outTg[g][:, s * 128:(s + 1) * 128],
                            ident,
                        )
                    if g % 2 == 0:
                        nc.vector.tensor_copy(out2g[g][:], tp[:])
                    else:
                        nc.gpsimd.tensor_copy(out2g[g][:], tp[:])
                    for cb in range(2):
                        py = psY.tile([128, 512], F32, tag="py")
                        nc.tensor.matmul(
                            py[:], woutT[:, cb * 128:(cb + 1) * 128], out2g[g][:],
                            start=True, stop=True,
                        )
                        dst = yg[cb][g][:]
                        if cb == 0:
                            nc.scalar.activation(dst, py[:], AF.Identity, bias=bo[cb])
                        else:
                            nc.vector.tensor_scalar_add(dst, py[:], bo[cb])
                        [nc.sync, nc.scalar][cb].dma_start(
                            out=y_ext[cb * 128:(cb + 1) * 128,
                                      g * 512:(g + 1) * 512],
                            in_=dst)

    nc.compile()
    return nc


def get_nc():
    if "nc" not in _CACHE:
        _CACHE["nc"] = build_nc()
    return _CACHE["nc"]


def make_in_maps(x, W_qkv, b_qkv, W_out, b_out):
    x = np.asarray(x, dtype=np.float32)
    W_qkv = np.asarray(W_qkv, dtype=np.float32)
    b_qkv = np.asarray(b_qkv, dtype=np.float32)
    W_out = np.asarray(W_out, dtype=np.float32) / SFIX   # undo Pn renorm
    b_out = np.asarray(b_out, dtype=np.float32)

    operm = (np.arange(O) + O // 2) % O      # rotate qkv channels by 192
    eperm = (np.arange(E) + E // 2) % E      # rotate e-axis by 64

    halves = []
    for h in range(2):
        if h == 0:
            wq, bqv, wo, bov = W_qkv, b_qkv, W_out, b_out
        else:
            wq = W_qkv[operm]
            bqv = b_qkv[operm]
            wo = W_out[:, eperm]
            bov = np.zeros_like(b_out)
        halves.append({
            "wqkvT": np.ascontiguousarray(wq.T).astype(ml_dtypes.bfloat16),
            "bqkv": np.ascontiguousarray(bqv.reshape(O, 1)),
            "woutT": np.ascontiguousarray(wo.T).astype(ml_dtypes.bfloat16),
            "bout": np.ascontiguousarray(bov.reshape(C, 1)),
        })

    xb = [np.ascontiguousarray(x[n].reshape(C, HW)).astype(ml_dtypes.bfloat16)
          for n in range(N)]
    in_maps = []
    for core in range(8):
        n, h = core // 2, core % 2
        m = {"x": xb[n]}
        m.update(halves[h])
        in_maps.append(m)
    return in_maps


def run(inputs, trace=False, **kw):
    nc = get_nc()
    in_maps = make_in_maps(**inputs)
    res = run_bass_kernel_spmd(nc, in_maps, core_ids=list(range(8)), trace=trace, **kw)
    ys = [np.asarray(res.results[i]["out"], dtype=np.float32) for i in range(8)]
    y = np.stack([ys[2 * n] + ys[2 * n + 1] for n in range(N)])
    return y.reshape(N, C, 64, 64), res


def kernel(**inputs):
    y, _ = run(inputs, trace=False)
    return y


# revision 5
# speedup vs baseline: 1.0307x; 1.0057x over previous
"""Trainium2 Bass kernel for nn_Attention (dense_transformer).

Reference computation (per batch n of 4):
  qkv = W_qkv @ x + b          (384, 4096)   [x flattened to (256, 64*64)]
  raw C-order reinterpret of qkv flat buffer as (4096, 384) -> q|k|v (4096,128) each
  scores = q @ k.T / 64        (4096, 4096)
  soft = softmax(scores, axis=-2)             [column softmax]
  out = soft @ v               (4096, 128)
  raw reinterpret of out as (128, 4096)
  y = W_out @ out2 + b_out     (256, 4096)

Sharding: 8 cores = 4 batches x 2 column-chunks (j-axis of the score
matrix = rows of k/v).  Column-softmax stats are local to a j-chunk;
each core produces a partial y, host sums the pair.  The SPMD graph is
identical on all cores; the j-half selection is encoded host-side by
rotating the qkv output channels by 192 for odd cores and rotating
W_out's e-axis by 64 to compensate.

Compute layout (per core), v3:
  head:    x loads + F = W_qkv@x+b (bf16) -> fbuf writes interleaved
           with the qT/kT xbar transposes on the sync ring (the DMA
           device is an exclusive serial resource - issue order is
           priority).  F drains on DVE/ACT(ob0)/Pool(ob1,2) so no
           queue blocks another.  v loads via SWDGE late (tail use).
  phase A: ACT runs ONLY exps (the critical path).  Lead chunks
           (jb0-3 x 1024-wide halves of i<2048) start as soon as
           qT part0/kT part0 land (~10us); then 2048-wide chunks
           (psA = 2 x 4 PSUM banks).  Each exp: 4 score MMs bf16 ->
           PSUM, exp with accum_out -> bf16 ring.
  stats:   DVE: zsum = reduce(zacc); szinv = 1/(zsum/4096) = 4096/Z.
  converts (DVE/Pool/ACT-last, hidden under ACT): dPn8 =
           fp8e4(P*szinv - 1) scattered into P8 so column i'=hb*128+e2
           holds score row i=e2*32+hb (proj2-transpose order).
           v8 = fp8e4(v) plain.
  tail:    colsum[e] = sum_j v_bf16[j,e] (16 trivial MMs; the rank-1
           "1" term of Pn = 1 + dPn); per 512-wide group g: 8 fp8
           DoubleRow pair-MMs (4x PE) accumulate dPn8 @ v8, drain
           with bias=colsum, 4 TensorE transposes, out2 copy, proj2
           (woutT pre-scaled by 1/4096 host-side) + b_out, y -> sync.
"""

import numpy as np
import ml_dtypes

import concourse.bass as bass
import concourse.bacc as bacc
import concourse.mybir as mybir
from concourse.bass_utils import run_bass_kernel_spmd
from concourse.tile import TileContext, add_dep_helper
from concourse.masks import make_identity

BF16 = mybir.dt.bfloat16
F32 = mybir.dt.float32
FP8 = mybir.dt.float8e4
AF = mybir.ActivationFunctionType
ALU = mybir.AluOpType
DR = mybir.MatmulPerfMode.DoubleRow

N, C, E, O, HW = 4, 256, 128, 384, 4096
JC = HW // 2          # j-chunk per core
NJB = JC // 128       # 16 j-blocks
SCALE = 1.0 / 64.0    # 1/sqrt(HW)
SFIX = 4096.0         # softmax renorm: Pn = P * (4096/Z), undone in W_out

_CACHE = {}


def build_nc():
    nc = bacc.Bacc("TRN2", target_bir_lowering=False, debug=False, num_devices=8)

    x_ext = nc.dram_tensor("x", [C, HW], BF16, kind="ExternalInput").ap()
    wqkvT_ext = nc.dram_tensor("wqkvT", [C, O], BF16, kind="ExternalInput").ap()
    bqkv_ext = nc.dram_tensor("bqkv", [O, 1], F32, kind="ExternalInput").ap()
    woutT_ext = nc.dram_tensor("woutT", [E, C], BF16, kind="ExternalInput").ap()
    bout_ext = nc.dram_tensor("bout", [C, 1], F32, kind="ExternalInput").ap()
    y_ext = nc.dram_tensor("out", [C, HW], BF16, kind="ExternalOutput").ap()

    fbuf = nc.dram_tensor("fbuf", [O * HW], BF16).ap()
    fview_o = fbuf.rearrange("(o hw) -> o hw", hw=HW)   # (384, 4096) write view
    fview_i = fbuf.rearrange("(i j) -> i j", j=O)        # (4096, 384) read view

    # persistent SBUF.  qT/kT/v split at 512-aligned boundaries covered by
    # successive F o-tiles.
    QSPL = [0, 1024, 2560, HW]       # parts covered by F o-tiles 0 / 0+1 / 1+2
    KSPL = [0, 1024, JC]
    VSPL = [0, 1280, JC]
    qTp = [nc.alloc_sbuf_tensor(f"qT{i}", [128, QSPL[i + 1] - QSPL[i]], BF16).ap()
           for i in range(3)]
    kTp = [nc.alloc_sbuf_tensor(f"kT{i}", [128, KSPL[i + 1] - KSPL[i]], BF16).ap()
           for i in range(2)]
    vp = [nc.alloc_sbuf_tensor(f"v{i}", [128, VSPL[i + 1] - VSPL[i]], BF16).ap()
          for i in range(2)]

    def qT_sl(i0, w=512):
        p = 0 if i0 < 1024 else (1 if i0 < 2560 else 2)
        a = i0 - QSPL[p]
        assert a + w <= QSPL[p + 1] - QSPL[p]
        return qTp[p][:, a:a + w]

    def kT_sl(jb):
        p = 0 if jb < 8 else 1
        a = jb * 128 - KSPL[p]
        return kTp[p][:, a:a + 128]

    def v_sl(jb):
        p = 0 if jb < 10 else 1
        a = jb * 128 - VSPL[p]
        return vp[p][:, a:a + 128]

    v8 = nc.alloc_sbuf_tensor("v8", [128, JC], FP8).ap()       # (j, e) fp8
    zacc = nc.alloc_sbuf_tensor("zacc", [128, 36], F32).ap()
    zsum = nc.alloc_sbuf_tensor("zsum", [128, 16], F32).ap()
    ztmp = nc.alloc_sbuf_tensor("ztmp", [128, 16], F32).ap()
    szinv = nc.alloc_sbuf_tensor("szinv", [128, 16], F32).ap()  # 4096/Z per jb
    cs_sb = nc.alloc_sbuf_tensor("cs_sb", [128, 1], F32).ap()   # colsum(v)
    outTg = [nc.alloc_sbuf_tensor(f"outTg{g}", [128, 512], BF16).ap()
             for g in range(8)]
    out2g = [nc.alloc_sbuf_tensor(f"out2g{g}", [128, 512], BF16).ap()
             for g in range(8)]
    # dPn8: per jb, column i' = hb*128 + e2 holds data for row i = e2*32 + hb.
    P8 = nc.alloc_sbuf_tensor("P8", [128, NJB * HW], FP8).ap()
    # bf16 exp staging: 8 dedicated lead buffers + 6-deep ring for the rest.
    Plead = nc.alloc_sbuf_tensor("Plead", [128, 8 * 1024], BF16).ap()
    PR = 6
    Pring = nc.alloc_sbuf_tensor("Pring", [128, PR * 2048], BF16).ap()

    # zacc slot map: jb<4 -> 3 chunks (A,B,C), jb>=4 -> 2 chunks (H0,H1)
    def zslot(jb, i0):
        if jb < 4:
            return 3 * jb + (0 if i0 == 0 else (1 if i0 == 1024 else 2))
        return 12 + 2 * (jb - 4) + (0 if i0 == 0 else 1)

    def nchunks(jb):
        return 3 if jb < 4 else 2

    def p8_dst(jb, i0, w):
        # out AP dims (e2: w/32 @1 offset i0/32, hb: 32 @128) within jb region
        reg = P8[:, jb * HW:(jb + 1) * HW]
        v = reg.rearrange("p (hb e) -> p e hb", hb=32)
        return v[:, i0 // 32:(i0 + w) // 32, :]

    def p8_rhs(t, g):
        # rhs for (pair t, group g): dims (p, jb-pair 2 @4096, 512 @1)
        reg = P8[:, t * 2 * HW:(t + 1) * 2 * HW]
        v = reg.rearrange("p (two x) -> p two x", two=2)
        return v[:, :, g * 512:(g + 1) * 512]

    with TileContext(nc) as tc:
        with tc.tile_pool(name="consts", bufs=1) as consts:
            # ---- constants (scalar ring; bias/wq gate stage-1 compute) ----
            bias = consts.tile([128, 8], F32, name="bias", tag="bias")
            bq = [bias[:, i:i + 1] for i in range(3)]
            bo = [bias[:, 3 + i:4 + i] for i in range(2)]
            for ob in range(3):
                nc.scalar.dma_start(out=bq[ob], in_=bqkv_ext[ob * 128:(ob + 1) * 128, :])
            wq_all = consts.tile([128, 2 * O], BF16, name="wq_all", tag="wq_all")
            wqT = [wq_all[:, 0:O], wq_all[:, O:2 * O]]
            for cb in range(2):
                nc.scalar.dma_start(out=wqT[cb], in_=wqkvT_ext[cb * 128:(cb + 1) * 128, :])
            for cb in range(2):
                nc.scalar.dma_start(out=bo[cb], in_=bout_ext[cb * 128:(cb + 1) * 128, :])
            misc = consts.tile([128, C + 128 + 8], BF16, name="misc", tag="misc")
            woutT = misc[:, 0:C]
            ident = misc[:, C:C + 128]
            ones16 = misc[:, C + 128:C + 129]
            nc.scalar.dma_start(out=woutT, in_=woutT_ext[:])
            make_identity(nc, ident)
            nc.vector.memset(ones16, 1.0)
            scratch = consts.tile([128, 2], F32, name="scratch", tag="scratch")
            neg1 = scratch[:, 1:2]
            nc.vector.memset(scratch[:, 0:1], 0.0)
            nc.vector.memset(neg1, -1.0)
            nc.scalar.activation(scratch[:, 0:1], scratch[:, 0:1], AF.Exp)

            # ---- PE warmup: dummy matmuls so HAM ramps early ----
            wsrc = consts.tile([128, 128], BF16, name="wsrc", tag="wsrc")
            nc.vector.memset(wsrc[:], 1.0)
            with tc.tile_pool(name="psW", bufs=1, space="PSUM") as psW:
                wtile = psW.tile([128, 128], F32, tag="warm")
                for _ in range(16):
                    nc.tensor.matmul(wtile[:], wsrc[:], wsrc[:], start=True, stop=True)

            # ---- x loads: all on sync ring (SP has nothing else to do) ----
            early = tc.alloc_tile_pool(name="early", bufs=1)
            xsb = [[early.tile([128, HW // 2], BF16, name=f"x{cb}{h}", tag=f"x{cb}{h}")
                    for h in range(2)] for cb in range(2)]
            Fsb = [[early.tile([128, HW // 2], BF16, name=f"F{i}{hh}", tag=f"F{i}{hh}")
                    for hh in range(2)] for i in range(3)]
            for h in range(2):
                for cb in range(2):
                    nc.sync.dma_start(
                        out=xsb[cb][h][:],
                        in_=x_ext[cb * 128:(cb + 1) * 128,
                                  h * (HW // 2):(h + 1) * (HW // 2)])

            # ---- stage 1: F o-tiles -> fbuf, transposes interleaved on the
            #      sync ring right after the F writes they depend on ----
            # drains: h0 chunks -> DVE; h1 chunks: ob0 -> ACT, ob1/ob2 -> Pool
            # (keeps ACT free for lead exps after ~7us).
            tr_specs = {
                "qT0": (qTp[0], fview_i[0:1024, 0:E], (0,)),
                "kT0": (kTp[0], fview_i[0:1024, E:2 * E], (0,)),
                "qT1": (qTp[1], fview_i[1024:2560, 0:E], (0, 1)),
                "kT1": (kTp[1], fview_i[1024:2048, E:2 * E], (0, 1)),
                "qT2": (qTp[2], fview_i[2560:HW, 0:E], (1, 2)),
            }
            with tc.tile_pool(name="psF", bufs=4, space="PSUM") as psF:
                f_writes = [[] for _ in range(3)]

                def launch_tr(key):
                    dst, srcap, deps = tr_specs[key]
                    rt = nc.sync.dma_start_transpose(out=dst[:], in_=srcap)
                    for dep in deps:
                        for w2 in f_writes[dep]:
                            add_dep_helper(rt.ins, w2.ins, reason="fbuf RAW")

                for ob in range(3):
                    for nch in range(8):
                        pf = psF.tile([128, 512], F32, tag="pf")
                        h, o512 = nch // 4, (nch % 4) * 512
                        nc.tensor.matmul(
                            pf[:], wqT[0][:, ob * 128:(ob + 1) * 128],
                            xsb[0][h][:, o512:o512 + 512],
                            start=True, stop=False,
                        )
                        nc.tensor.matmul(
                            pf[:], wqT[1][:, ob * 128:(ob + 1) * 128],
                            xsb[1][h][:, o512:o512 + 512],
                            start=False, stop=True,
                        )
                        fb = Fsb[ob][nch // 4]
                        fsl = slice((nch % 4) * 512, (nch % 4 + 1) * 512)
                        if nch < 4:
                            nc.vector.tensor_scalar_add(fb[:, fsl], pf[:], bq[ob])
                        elif ob == 0:
                            nc.scalar.activation(fb[:, fsl], pf[:], AF.Identity,
                                                 bias=bq[ob])
                        else:
                            nc.gpsimd.tensor_scalar_add(fb[:, fsl], pf[:], bq[ob])
                        if nch == 3 or nch == 7:
                            hh = nch // 4
                            f_writes[ob].append(nc.sync.dma_start(
                                out=fview_o[ob * 128:(ob + 1) * 128,
                                            hh * (HW // 2):(hh + 1) * (HW // 2)],
                                in_=Fsb[ob][hh][:],
                            ))
                    if ob == 0:
                        launch_tr("qT0")
                        launch_tr("kT0")
                    elif ob == 1:
                        launch_tr("qT1")
                        launch_tr("kT1")
                    else:
                        launch_tr("qT2")
                    # v loads (SWDGE) after their covering F writes
                    if ob == 0:
                        r = nc.gpsimd.dma_start(
                            out=vp[0].rearrange("p (t d) -> p t d", d=128),
                            in_=fview_i[0:1280, 2 * E:3 * E].rearrange(
                                "(t p) d -> p t d", p=128))
                        for w2 in f_writes[0]:
                            add_dep_helper(r.ins, w2.ins, reason="fbuf RAW")
                    elif ob == 1:
                        r = nc.gpsimd.dma_start(
                            out=vp[1].rearrange("p (t d) -> p t d", d=128),
                            in_=fview_i[1280:2048, 2 * E:3 * E].rearrange(
                                "(t p) d -> p t d", p=128))
                        for w2 in f_writes[0] + f_writes[1]:
                            add_dep_helper(r.ins, w2.ins, reason="fbuf RAW")
            early.release()

            # v8 = fp8(v) plain converts (DVE), as soon as v parts land
            nc.vector.tensor_copy(v8[:, 0:1280], vp[0][:])
            nc.vector.tensor_copy(v8[:, 1280:JC], vp[1][:])

            # ---- phase A ----
            def score_exp(pool, jb, i0, w, buf):
                pa = pool.tile([128, w], F32, tag="pa")
                for n2 in range(w // 512):
                    nc.tensor.matmul(
                        pa[:, n2 * 512:(n2 + 1) * 512],
                        kT_sl(jb), qT_sl(i0 + n2 * 512),
                        start=True, stop=True,
                    )
                zs = zslot(jb, i0)
                nc.scalar.activation(
                    out=buf, in_=pa[:], func=AF.Exp, scale=SCALE,
                    accum_out=zacc[:, zs:zs + 1],
                )

            def stats(jb):
                k = nchunks(jb)
                s0 = zslot(jb, 0)
                nc.vector.reduce_sum(
                    out=zsum[:, jb:jb + 1], in_=zacc[:, s0:s0 + k],
                    axis=mybir.AxisListType.X,
                )
                nc.vector.tensor_scalar_mul(
                    ztmp[:, jb:jb + 1], zsum[:, jb:jb + 1], 1.0 / SFIX)
                nc.vector.reciprocal(szinv[:, jb:jb + 1], ztmp[:, jb:jb + 1])

            def convert(jb, i0, w, buf, eng):
                src = buf.rearrange("p (e hb) -> p e hb", hb=32)
                dst = p8_dst(jb, i0, w)
                if eng is nc.scalar:
                    nc.scalar.activation(dst, src, AF.Identity,
                                         scale=szinv[:, jb:jb + 1], bias=neg1)
                else:
                    eng.tensor_scalar(
                        out=dst, in0=src,
                        scalar1=szinv[:, jb:jb + 1], scalar2=1.0,
                        op0=ALU.mult, op1=ALU.subtract,
                    )

            def conv_eng(jb):
                return nc.gpsimd if 4 <= jb < 12 else nc.vector

            # lead chunks: jb0-3, i<2048 as 1024-wide halves (only need
            # qT part0/1 + kT part0); dedicated buffers (no ring WAR).
            with tc.tile_pool(name="psL", bufs=2, space="PSUM") as psL:
                for li, (jb, i0) in enumerate(
                        [(j, 0) for j in range(4)] + [(j, 1024) for j in range(4)]):
                    buf = Plead[:, li * 1024:(li + 1) * 1024]
                    score_exp(psL, jb, i0, 1024, buf)

            # main chunks (2048 wide): jb0-3 C first, then jb4-15 pairs.
            mains = [(jb, 2048) for jb in range(4)]
            for jb in range(4, NJB):
                mains += [(jb, 0), (jb, 2048)]
            with tc.tile_pool(name="psA", bufs=2, space="PSUM") as psA:
                for seq, (jb, i0) in enumerate(mains):
                    buf = Pring[:, (seq % PR) * 2048:(seq % PR + 1) * 2048]
                    score_exp(psA, jb, i0, 2048, buf)
                    if i0 == 2048:       # jb complete -> stats + converts
                        stats(jb)
                        if jb < 4:
                            lA = Plead[:, (jb) * 1024:(jb + 1) * 1024]
                            lB = Plead[:, (4 + jb) * 1024:(5 + jb) * 1024]
                            convert(jb, 0, 1024, lA, nc.vector)
                            convert(jb, 1024, 1024, lB, nc.vector)
                            convert(jb, 2048, 2048, buf, nc.vector)
                        else:
                            h0buf = Pring[:, ((seq - 1) % PR) * 2048:
                                          ((seq - 1) % PR + 1) * 2048]
                            e0 = nc.scalar if jb == 15 else conv_eng(jb)
                            convert(jb, 0, 2048, h0buf, e0)
                            convert(jb, 2048, 2048, buf,
                                    nc.vector if jb >= 12 else conv_eng(jb))

            # ---- tail: colsum, fp8 DoubleRow out-MMs, transpose, proj2 ----
            with tc.tile_pool(name="psCS", bufs=1, space="PSUM") as psCS, \
                 tc.tile_pool(name="psB", bufs=2, space="PSUM") as psB, \
                 tc.tile_pool(name="psC", bufs=2, space="PSUM") as psC, \
                 tc.tile_pool(name="psY", bufs=2, space="PSUM") as psY, \
                 tc.tile_pool(name="late", bufs=1) as late:
                # colsum[e] = sum_j v[j, e] in bf16 (rank-1 term of Pn)
                cs_ps = psCS.tile([128, 1], F32, tag="cs")
                for jb in range(NJB):
                    nc.tensor.matmul(
                        cs_ps[:], v_sl(jb), ones16,
                        start=(jb == 0), stop=(jb == NJB - 1),
                    )
                nc.vector.tensor_copy(cs_sb[:], cs_ps[:])

                yg = [[late.tile([128, 512], BF16, name=f"yb{cb}_{g}",
                                 tag=f"yb{cb}_{g}") for g in range(8)]
                      for cb in range(2)]
                for g in range(8):
                    ob_ps = psB.tile([128, 512], F32, tag="ob_ps")
                    for t in range(NJB // 2):
                        v8pair = v8[:, t * 256:(t + 1) * 256].rearrange(
                            "p (two e) -> p two e", two=2)
                        nc.tensor.matmul(
                            ob_ps[:], v8pair, p8_rhs(t, g),
                            start=(t == 0), stop=(t == NJB // 2 - 1),
                            perf_mode=DR,
                        )
                    if g % 2 == 0:
                        nc.scalar.activation(outTg[g][:], ob_ps[:], AF.Identity,
                                             bias=cs_sb[:])
                    else:
                        nc.vector.tensor_scalar_add(outTg[g][:], ob_ps[:], cs_sb[:])
                    tp = psC.tile([128, 512], BF16, tag="tp")
                    for s in range(4):
                        nc.tensor.transpose(
                            tp[:, s * 128:(s + 1) * 128],
                            outTg[g][:, s * 128:(s + 1) * 128],
                            ident,
                        )
                    if g % 2 == 0:
                        nc.vector.tensor_copy(out2g[g][:], tp[:])
                    else:
                        nc.gpsimd.tensor_copy(out2g[g][:], tp[:])
                    for cb in range(2):
                        py = psY.tile([128, 512], F32, tag="py")
                        nc.tensor.matmul(
                            py[:], woutT[:, cb * 128:(cb + 1) * 128], out2g[g][:],
                            start=True, stop=True,
                        )
                        dst = yg[cb][g][:]
                        if cb == 0:
                            nc.scalar.activation(dst, py[:], AF.Identity, bias=bo[cb])
                        else:
                            nc.vector.tensor_scalar_add(dst, py[:], bo[cb])
                        nc.sync.dma_start(
                            out=y_ext[cb * 128:(cb + 1) * 128,
                                      g * 512:(g + 1) * 512],
                            in_=dst)

    nc.compile()
    return nc


def get_nc():
    if "nc" not in _CACHE:
        _CACHE["nc"] = build_nc()
    return _CACHE["nc"]


def make_in_maps(x, W_qkv, b_qkv, W_out, b_out):
    x = np.asarray(x, dtype=np.float32)
    W_qkv = np.asarray(W_qkv, dtype=np.float32)
    b_qkv = np.asarray(b_qkv, dtype=np.float32)
    W_out = np.asarray(W_out, dtype=np.float32) / SFIX   # undo Pn renorm
    b_out = np.asarray(b_out, dtype=np.float32)

    operm = (np.arange(O) + O // 2) % O      # rotate qkv channels by 192
    eperm = (np.arange(E) + E // 2) % E      # rotate e-axis by 64

    halves = []
    for h in range(2):
        if h == 0:
            wq, bqv, wo, bov = W_qkv, b_qkv, W_out, b_out
        else:
            wq = W_qkv[operm]
            bqv = b_qkv[operm]
            wo = W_out[:, eperm]
            bov = np.zeros_like(b_out)
        halves.append({
            "wqkvT": np.ascontiguousarray(wq.T).astype(ml_dtypes.bfloat16),
            "bqkv": np.ascontiguousarray(bqv.reshape(O, 1)),
            "woutT": np.ascontiguousarray(wo.T).astype(ml_dtypes.bfloat16),
            "bout": np.ascontiguousarray(bov.reshape(C, 1)),
        })

    xb = [np.ascontiguousarray(x[n].reshape(C, HW)).astype(ml_dtypes.bfloat16)
          for n in range(N)]
    in_maps = []
    for core in range(8):
        n, h = core // 2, core % 2
        m = {"x": xb[n]}
        m.update(halves[h])
        in_maps.append(m)
    return in_maps


def run(inputs, trace=False, **kw):
    nc = get_nc()
    in_maps = make_in_maps(**inputs)
    res = run_bass_kernel_spmd(nc, in_maps, core_ids=list(range(8)), trace=trace, **kw)
    ys = [np.asarray(res.results[i]["out"], dtype=np.float32) for i in range(8)]
    y = np.stack([ys[2 * n] + ys[2 * n + 1] for n in range(N)])
    return y.reshape(N, C, 64, 64), res


def kernel(**inputs):
    y, _ = run(inputs, trace=False)
    return y


# revision 13
# speedup vs baseline: 1.1207x; 1.0873x over previous
"""Trainium2 Bass kernel for nn_Attention (dense_transformer).

Reference computation (per batch n of 4):
  qkv = W_qkv @ x + b          (384, 4096)   [x flattened to (256, 64*64)]
  raw C-order reinterpret of qkv flat buffer as (4096, 384) -> q|k|v (4096,128) each
  scores = q @ k.T / 64        (4096, 4096)
  soft = softmax(scores, axis=-2)             [column softmax]
  out = soft @ v               (4096, 128)
  raw reinterpret of out as (128, 4096)
  y = W_out @ out2 + b_out     (256, 4096)

Sharding: 8 cores = 4 batches x 2 column-chunks (j-axis of the score
matrix = rows of k/v).  Column-softmax stats are local to a j-chunk;
each core produces a partial y, host sums the pair.  The SPMD graph is
identical on all cores; the j-half selection is encoded host-side by
rotating the qkv output channels by 192 for odd cores and rotating
W_out's e-axis by 64 to compensate.

Compute layout (per core), v3:
  head:    x loads + F = W_qkv@x+b (bf16) -> fbuf writes interleaved
           with the qT/kT xbar transposes on the sync ring (the DMA
           device is an exclusive serial resource - issue order is
           priority).  F drains on DVE/ACT(ob0)/Pool(ob1,2) so no
           queue blocks another.  v loads via SWDGE late (tail use).
  phase A: ACT runs ONLY exps (the critical path).  Lead chunks
           (jb0-3 x 1024-wide halves of i<2048) start as soon as
           qT part0/kT part0 land (~10us); then 2048-wide chunks
           (psA = 2 x 4 PSUM banks).  Each exp: 4 score MMs bf16 ->
           PSUM, exp with accum_out -> bf16 ring.
  stats:   DVE: zsum = reduce(zacc); szinv = 1/(zsum/4096) = 4096/Z.
  converts (DVE/Pool/ACT-last, hidden under ACT): dPn8 =
           fp8e4(P*szinv - 1) scattered into P8 so column i'=hb*128+e2
           holds score row i=e2*32+hb (proj2-transpose order).
           v8 = fp8e4(v) plain.
  tail:    colsum[e] = sum_j v_bf16[j,e] (16 trivial MMs; the rank-1
           "1" term of Pn = 1 + dPn); per 512-wide group g: 8 fp8
           DoubleRow pair-MMs (4x PE) accumulate dPn8 @ v8, drain
           with bias=colsum, 4 TensorE transposes, out2 copy, proj2
           (woutT pre-scaled by 1/4096 host-side) + b_out, y -> sync.
"""

import numpy as np
import ml_dtypes

import concourse.bass as bass
import concourse.bacc as bacc
import concourse.mybir as mybir
from concourse.bass_utils import run_bass_kernel_spmd
from concourse.tile import TileContext, add_dep_helper
from concourse.masks import make_identity

BF16 = mybir.dt.bfloat16
F32 = mybir.dt.float32
FP8 = mybir.dt.float8e4
AF = mybir.ActivationFunctionType
ALU = mybir.AluOpType
DR = mybir.MatmulPerfMode.DoubleRow

N, C, E, O, HW = 4, 256, 128, 384, 4096
JC = HW // 2          # j-chunk per core
NJB = JC // 128       # 16 j-blocks
SCALE = 1.0 / 64.0    # 1/sqrt(HW)
SFIX = 4096.0         # softmax renorm: Pn = P * (4096/Z), undone in W_out

_CACHE = {}


def build_nc():
    nc = bacc.Bacc("TRN2", target_bir_lowering=False, debug=False, num_devices=8)

    x_ext = nc.dram_tensor("x", [C, HW], BF16, kind="ExternalInput").ap()
    wqkvT_ext = nc.dram_tensor("wqkvT", [C, O], BF16, kind="ExternalInput").ap()
    bqkv_ext = nc.dram_tensor("bqkv", [O, 1], F32, kind="ExternalInput").ap()
    woutT_ext = nc.dram_tensor("woutT", [E, C], BF16, kind="ExternalInput").ap()
    bout_ext = nc.dram_tensor("bout", [C, 1], F32, kind="ExternalInput").ap()
    y_ext = nc.dram_tensor("out", [C, HW], BF16, kind="ExternalOutput").ap()

    fbuf = nc.dram_tensor("fbuf", [O * HW], BF16).ap()
    fview_o = fbuf.rearrange("(o hw) -> o hw", hw=HW)   # (384, 4096) write view
    fview_i = fbuf.rearrange("(i j) -> i j", j=O)        # (4096, 384) read view

    # persistent SBUF.  qT/kT/v split at 512-aligned boundaries covered by
    # successive F o-tiles.
    QSPL = [0, 1024, 2560, HW]       # parts covered by F o-tiles 0 / 0+1 / 1+2
    KSPL = [0, 1024, JC]
    VSPL = [0, 1280, JC]
    qTp = [nc.alloc_sbuf_tensor(f"qT{i}", [128, QSPL[i + 1] - QSPL[i]], BF16).ap()
           for i in range(3)]
    kTp = [nc.alloc_sbuf_tensor(f"kT{i}", [128, KSPL[i + 1] - KSPL[i]], BF16).ap()
           for i in range(2)]
    vp = [nc.alloc_sbuf_tensor(f"v{i}", [128, VSPL[i + 1] - VSPL[i]], BF16).ap()
          for i in range(2)]

    def qT_sl(i0, w=512):
        p = 0 if i0 < 1024 else (1 if i0 < 2560 else 2)
        a = i0 - QSPL[p]
        assert a + w <= QSPL[p + 1] - QSPL[p]
        return qTp[p][:, a:a + w]

    def kT_sl(jb):
        p = 0 if jb < 8 else 1
        a = jb * 128 - KSPL[p]
        return kTp[p][:, a:a + 128]

    def v_sl(jb):
        p = 0 if jb < 10 else 1
        a = jb * 128 - VSPL[p]
        return vp[p][:, a:a + 128]

    v8 = nc.alloc_sbuf_tensor("v8", [128, JC], FP8).ap()       # (j, e) fp8
    zacc = nc.alloc_sbuf_tensor("zacc", [128, 36], F32).ap()
    zsum = nc.alloc_sbuf_tensor("zsum", [128, 16], F32).ap()
    ztmp = nc.alloc_sbuf_tensor("ztmp", [128, 16], F32).ap()
    szinv = nc.alloc_sbuf_tensor("szinv", [128, 16], F32).ap()  # 4096/Z per jb
    cs_row = nc.alloc_sbuf_tensor("cs_row", [1, 128], BF16).ap()  # colsum(v) row
    out2g = [nc.alloc_sbuf_tensor(f"out2g{g}", [128, 512], BF16).ap()
             for g in range(8)]
    # dPn8: per jb, column i' = hb*128 + e2 holds data for row i = e2*32 + hb.
    P8 = nc.alloc_sbuf_tensor("P8", [128, NJB * HW], FP8).ap()
    # bf16 exp staging: 8 dedicated lead buffers + 6-deep ring for the rest.
    Plead = nc.alloc_sbuf_tensor("Plead", [128, 8 * 1024], BF16).ap()
    PR = 6
    Pring = nc.alloc_sbuf_tensor("Pring", [128, PR * 2048], BF16).ap()

    # zacc slot map: jb<4 -> 3 chunks (A,B,C), jb>=4 -> 2 chunks (H0,H1)
    def zslot(jb, i0):
        if jb < 4:
            return 3 * jb + (0 if i0 == 0 else (1 if i0 == 1024 else 2))
        return 12 + 2 * (jb - 4) + (0 if i0 == 0 else 1)

    def nchunks(jb):
        return 3 if jb < 4 else 2

    def p8_dst(jb, i0, w):
        # out AP dims (e2: w/32 @1 offset i0/32, hb: 32 @128) within jb region
        reg = P8[:, jb * HW:(jb + 1) * HW]
        v = reg.rearrange("p (hb e) -> p e hb", hb=32)
        return v[:, i0 // 32:(i0 + w) // 32, :]

    def p8_lhsT(t, hb):
        # stationary for (pair t, hb-block): dims (p, jb-pair 2 @4096, 128 @1)
        reg = P8[:, t * 2 * HW:(t + 1) * 2 * HW]
        v = reg.rearrange("p (two x) -> p two x", two=2)
        return v[:, :, hb * 128:(hb + 1) * 128]

    with TileContext(nc) as tc:
        with tc.tile_pool(name="consts", bufs=1) as consts:
            # ---- constants (scalar ring; bias/wq gate stage-1 compute) ----
            bias = consts.tile([128, 8], F32, name="bias", tag="bias")
            bq = [bias[:, i:i + 1] for i in range(3)]
            bo = [bias[:, 3 + i:4 + i] for i in range(2)]
            for ob in range(3):
                nc.scalar.dma_start(out=bq[ob], in_=bqkv_ext[ob * 128:(ob + 1) * 128, :])
            wq_all = consts.tile([128, 2 * O], BF16, name="wq_all", tag="wq_all")
            wqT = [wq_all[:, 0:O], wq_all[:, O:2 * O]]
            for cb in range(2):
                nc.scalar.dma_start(out=wqT[cb], in_=wqkvT_ext[cb * 128:(cb + 1) * 128, :])
            for cb in range(2):
                nc.scalar.dma_start(out=bo[cb], in_=bout_ext[cb * 128:(cb + 1) * 128, :])
            misc = consts.tile([128, C + 8], BF16, name="misc", tag="misc")
            woutT = misc[:, 0:C]
            ones16 = misc[:, C:C + 1]
            nc.scalar.dma_start(out=woutT, in_=woutT_ext[:])
            nc.vector.memset(ones16, 1.0)
            scratch = consts.tile([128, 2], F32, name="scratch", tag="scratch")
            neg1 = scratch[:, 1:2]
            nc.vector.memset(scratch[:, 0:1], 0.0)
            nc.vector.memset(neg1, -1.0)
            nc.scalar.activation(scratch[:, 0:1], scratch[:, 0:1], AF.Exp)

            # ---- PE warmup: dummy matmuls so HAM ramps early ----
            wsrc = consts.tile([128, 128], BF16, name="wsrc", tag="wsrc")
            nc.vector.memset(wsrc[:], 1.0)
            with tc.tile_pool(name="psW", bufs=1, space="PSUM") as psW:
                wtile = psW.tile([128, 128], F32, tag="warm")
                for _ in range(16):
                    nc.tensor.matmul(wtile[:], wsrc[:], wsrc[:], start=True, stop=True)

            # ---- x loads: all on sync ring (SP has nothing else to do) ----
            early = tc.alloc_tile_pool(name="early", bufs=1)
            xsb = [[early.tile([128, HW // 2], BF16, name=f"x{cb}{h}", tag=f"x{cb}{h}")
                    for h in range(2)] for cb in range(2)]
            Fsb = [[early.tile([128, HW // 2], BF16, name=f"F{i}{hh}", tag=f"F{i}{hh}")
                    for hh in range(2)] for i in range(3)]
            for h in range(2):
                for cb in range(2):
                    nc.sync.dma_start(
                        out=xsb[cb][h][:],
                        in_=x_ext[cb * 128:(cb + 1) * 128,
                                  h * (HW // 2):(h + 1) * (HW // 2)])

            # ---- stage 1: F o-tiles -> fbuf, transposes interleaved on the
            #      sync ring right after the F writes they depend on ----
            # drains: h0 chunks -> DVE; h1 chunks: ob0 -> ACT, ob1/ob2 -> Pool
            # (keeps ACT free for lead exps after ~7us).
            tr_specs = {
                "qT0": (qTp[0], fview_i[0:1024, 0:E], (0,)),
                "kT0": (kTp[0], fview_i[0:1024, E:2 * E], (0,)),
                "qT1": (qTp[1], fview_i[1024:2560, 0:E], (0, 1)),
                "kT1": (kTp[1], fview_i[1024:2048, E:2 * E], (0, 1)),
                "qT2": (qTp[2], fview_i[2560:HW, 0:E], (1, 2)),
            }
            # psL (lead-chunk PSUM) opened BEFORE psF so its banks are
            # disjoint from stage-1's: no anti-deps delaying the lead exps.
            psL = tc.alloc_tile_pool(name="psL", bufs=2, space="PSUM")
            with tc.tile_pool(name="psF", bufs=4, space="PSUM") as psF:
                f_writes = [[] for _ in range(3)]

                def launch_tr(key):
                    dst, srcap, deps = tr_specs[key]
                    rt = nc.sync.dma_start_transpose(out=dst[:], in_=srcap)
                    for dep in deps:
                        for w2 in f_writes[dep]:
                            add_dep_helper(rt.ins, w2.ins, reason="fbuf RAW")

                for ob in range(3):
                    for nch in range(8):
                        pf = psF.tile([128, 512], F32, tag="pf")
                        h, o512 = nch // 4, (nch % 4) * 512
                        nc.tensor.matmul(
                            pf[:], wqT[0][:, ob * 128:(ob + 1) * 128],
                            xsb[0][h][:, o512:o512 + 512],
                            start=True, stop=False,
                        )
                        nc.tensor.matmul(
                            pf[:], wqT[1][:, ob * 128:(ob + 1) * 128],
                            xsb[1][h][:, o512:o512 + 512],
                            start=False, stop=True,
                        )
                        fb = Fsb[ob][nch // 4]
                        fsl = slice((nch % 4) * 512, (nch % 4 + 1) * 512)
                        if nch < 4:
                            nc.vector.tensor_scalar_add(fb[:, fsl], pf[:], bq[ob])
                        elif ob == 0:
                            nc.scalar.activation(fb[:, fsl], pf[:], AF.Identity,
                                                 bias=bq[ob])
                        else:
                            nc.gpsimd.tensor_scalar_add(fb[:, fsl], pf[:], bq[ob])
                        if nch == 3 or nch == 7:
                            hh = nch // 4
                            f_writes[ob].append(nc.sync.dma_start(
                                out=fview_o[ob * 128:(ob + 1) * 128,
                                            hh * (HW // 2):(hh + 1) * (HW // 2)],
                                in_=Fsb[ob][hh][:],
                            ))
                    if ob == 0:
                        launch_tr("qT0")
                        launch_tr("kT0")
                    elif ob == 1:
                        launch_tr("qT1")
                        launch_tr("kT1")
                    else:
                        launch_tr("qT2")
                # v loads (SWDGE) issued last: only needed in the tail, and
                # their gather transfers must not delay the transposes on the
                # serialized DMA device.
                for part, (r0, r1) in enumerate([(0, 1280), (1280, 2048)]):
                    r = nc.gpsimd.dma_start(
                        out=vp[part].rearrange("p (t d) -> p t d", d=128),
                        in_=fview_i[r0:r1, 2 * E:3 * E].rearrange(
                            "(t p) d -> p t d", p=128))
                    for w2 in f_writes[0] + f_writes[1]:
                        add_dep_helper(r.ins, w2.ins, reason="fbuf RAW")
            early.release()

            # v8 = fp8(v) plain converts (DVE), as soon as v parts land
            nc.vector.tensor_copy(v8[:, 0:1280], vp[0][:])
            nc.vector.tensor_copy(v8[:, 1280:JC], vp[1][:])

            # ---- phase A ----
            def score_exp(pool, jb, i0, w, buf):
                pa = pool.tile([128, w], F32, tag="pa")
                for n2 in range(w // 512):
                    nc.tensor.matmul(
                        pa[:, n2 * 512:(n2 + 1) * 512],
                        kT_sl(jb), qT_sl(i0 + n2 * 512),
                        start=True, stop=True,
                    )
                zs = zslot(jb, i0)
                nc.scalar.activation(
                    out=buf, in_=pa[:], func=AF.Exp, scale=SCALE,
                    accum_out=zacc[:, zs:zs + 1],
                )

            def stats(jb):
                k = nchunks(jb)
                s0 = zslot(jb, 0)
                nc.vector.reduce_sum(
                    out=zsum[:, jb:jb + 1], in_=zacc[:, s0:s0 + k],
                    axis=mybir.AxisListType.X,
                )
                nc.vector.tensor_scalar_mul(
                    ztmp[:, jb:jb + 1], zsum[:, jb:jb + 1], 1.0 / SFIX)
                nc.vector.reciprocal(szinv[:, jb:jb + 1], ztmp[:, jb:jb + 1])

            def convert(jb, i0, w, buf, eng):
                src = buf.rearrange("p (e hb) -> p e hb", hb=32)
                dst = p8_dst(jb, i0, w)
                if eng is nc.scalar:
                    nc.scalar.activation(dst, src, AF.Identity,
                                         scale=szinv[:, jb:jb + 1], bias=neg1)
                else:
                    eng.tensor_scalar(
                        out=dst, in0=src,
                        scalar1=szinv[:, jb:jb + 1], scalar2=1.0,
                        op0=ALU.mult, op1=ALU.subtract,
                    )

            def conv_eng(jb):
                # alternate DVE/Pool so neither falls behind the exp stream
                return nc.gpsimd if (jb >= 4 and jb % 2 == 0) else nc.vector

            # lead chunks: jb0-3, i<2048 as 1024-wide halves (only need
            # qT part0/1 + kT part0); dedicated buffers (no ring WAR).
            for li, (jb, i0) in enumerate(
                    [(j, 0) for j in range(4)] + [(j, 1024) for j in range(4)]):
                buf = Plead[:, li * 1024:(li + 1) * 1024]
                score_exp(psL, jb, i0, 1024, buf)
            psL.release()

            # main chunks (2048 wide): jb0-3 C first, then jb4-15 pairs.
            mains = [(jb, 2048) for jb in range(4)]
            for jb in range(4, NJB):
                mains += [(jb, 0), (jb, 2048)]
            with tc.tile_pool(name="psA", bufs=2, space="PSUM") as psA:
                for seq, (jb, i0) in enumerate(mains):
                    buf = Pring[:, (seq % PR) * 2048:(seq % PR + 1) * 2048]
                    score_exp(psA, jb, i0, 2048, buf)
                    if i0 == 2048:       # jb complete -> stats + converts
                        stats(jb)
                        if jb < 4:
                            lA = Plead[:, (jb) * 1024:(jb + 1) * 1024]
                            lB = Plead[:, (4 + jb) * 1024:(5 + jb) * 1024]
                            convert(jb, 0, 1024, lA, nc.vector)
                            convert(jb, 1024, 1024, lB, nc.vector)
                            convert(jb, 2048, 2048, buf, nc.vector)
                        else:
                            h0buf = Pring[:, ((seq - 1) % PR) * 2048:
                                          ((seq - 1) % PR + 1) * 2048]
                            e0 = nc.scalar if jb == 15 else conv_eng(jb)
                            convert(jb, 0, 2048, h0buf, e0)
                            convert(jb, 2048, 2048, buf, conv_eng(jb))

            # ---- tail: out2 produced DIRECTLY by fp8 DoubleRow pair-MMs
            #      (P8 stationary): out2[e2, hb*128+e] = sum_j Pn[j,..]*v8[j,e].
            #      The rank-1 "1" term (colsum row) seeds each accumulation. ----
            with tc.tile_pool(name="psCS", bufs=1, space="PSUM") as psCS, \
                 tc.tile_pool(name="psB", bufs=2, space="PSUM") as psB, \
                 tc.tile_pool(name="psY", bufs=2, space="PSUM") as psY, \
                 tc.tile_pool(name="late", bufs=1) as late:
                # colsum row: cs[0, e] = sum_j v[j, e] in bf16
                cs_ps = psCS.tile([1, 128], F32, tag="cs")
                for jb in range(NJB):
                    nc.tensor.matmul(
                        cs_ps[:], ones16, v_sl(jb),
                        start=(jb == 0), stop=(jb == NJB - 1),
                    )
                nc.vector.tensor_copy(cs_row[:], cs_ps[:])
                ones_row = wsrc[0:1, :]

                yg = [[late.tile([128, 512], BF16, name=f"yb{cb}_{g}",
                                 tag=f"yb{cb}_{g}") for g in range(8)]
                      for cb in range(2)]
                for g in range(8):
                    ob_ps = psB.tile([128, 512], F32, tag="ob_ps")
                    for s in range(4):
                        hb = 4 * g + s
                        sl = ob_ps[:, s * 128:(s + 1) * 128]
                        nc.tensor.matmul(
                            sl, ones_row, cs_row[:],
                            start=True, stop=False,
                        )
                        for t in range(NJB // 2):
                            v8pair = v8[:, t * 256:(t + 1) * 256].rearrange(
                                "p (two e) -> p two e", two=2)
                            nc.tensor.matmul(
                                sl, p8_lhsT(t, hb), v8pair,
                                start=False, stop=(t == NJB // 2 - 1),
                                perf_mode=DR,
                            )
                    if g % 3 == 0:
                        nc.scalar.activation(out2g[g][:], ob_ps[:], AF.Identity)
                    elif g % 3 == 1:
                        nc.vector.tensor_copy(out2g[g][:], ob_ps[:])
                    else:
                        nc.gpsimd.tensor_copy(out2g[g][:], ob_ps[:])
                    for cb in range(2):
                        py = psY.tile([128, 512], F32, tag="py")
                        nc.tensor.matmul(
                            py[:], woutT[:, cb * 128:(cb + 1) * 128], out2g[g][:],
                            start=True, stop=True,
                        )
                        dst = yg[cb][g][:]
                        if cb == 0:
                            nc.scalar.activation(dst, py[:], AF.Identity, bias=bo[cb])
                        else:
                            nc.vector.tensor_scalar_add(dst, py[:], bo[cb])
                        nc.sync.dma_start(
                            out=y_ext[cb * 128:(cb + 1) * 128,
                                      g * 512:(g + 1) * 512],
                            in_=dst)

    nc.compile()
    return nc


def get_nc():
    if "nc" not in _CACHE:
        _CACHE["nc"] = build_nc()
    return _CACHE["nc"]


def make_in_maps(x, W_qkv, b_qkv, W_out, b_out):
    x = np.asarray(x, dtype=np.float32)
    W_qkv = np.asarray(W_qkv, dtype=np.float32)
    b_qkv = np.asarray(b_qkv, dtype=np.float32)
    W_out = np.asarray(W_out, dtype=np.float32) / SFIX   # undo Pn renorm
    b_out = np.asarray(b_out, dtype=np.float32)

    operm = (np.arange(O) + O // 2) % O      # rotate qkv channels by 192
    eperm = (np.arange(E) + E // 2) % E      # rotate e-axis by 64

    halves = []
    for h in range(2):
        if h == 0:
            wq, bqv, wo, bov = W_qkv, b_qkv, W_out, b_out
        else:
            wq = W_qkv[operm]
            bqv = b_qkv[operm]
            wo = W_out[:, eperm]
            bov = np.zeros_like(b_out)
        halves.append({
            "wqkvT": np.ascontiguousarray(wq.T).astype(ml_dtypes.bfloat16),
            "bqkv": np.ascontiguousarray(bqv.reshape(O, 1)),
            "woutT": np.ascontiguousarray(wo.T).astype(ml_dtypes.bfloat16),
            "bout": np.ascontiguousarray(bov.reshape(C, 1)),
        })

    xb = [np.ascontiguousarray(x[n].reshape(C, HW)).astype(ml_dtypes.bfloat16)
          for n in range(N)]
    in_maps = []
    for core in range(8):
        n, h = core // 2, core % 2
        m = {"x": xb[n]}
        m.update(halves[h])
        in_maps.append(m)
    return in_maps


def run(inputs, trace=False, **kw):
    nc = get_nc()
    in_maps = make_in_maps(**inputs)
    res = run_bass_kernel_spmd(nc, in_maps, core_ids=list(range(8)), trace=trace, **kw)
    ys = [np.asarray(res.results[i]["out"], dtype=np.float32) for i in range(8)]
    y = np.stack([ys[2 * n] + ys[2 * n + 1] for n in range(N)])
    return y.reshape(N, C, 64, 64), res


def kernel(**inputs):
    y, _ = run(inputs, trace=False)
    return y


# revision 17
# speedup vs baseline: 1.1287x; 1.0072x over previous
"""Trainium2 Bass kernel for nn_Attention (dense_transformer).

Reference computation (per batch n of 4):
  qkv = W_qkv @ x + b          (384, 4096)   [x flattened to (256, 64*64)]
  raw C-order reinterpret of qkv flat buffer as (4096, 384) -> q|k|v (4096,128) each
  scores = q @ k.T / 64        (4096, 4096)
  soft = softmax(scores, axis=-2)             [column softmax]
  out = soft @ v               (4096, 128)
  raw reinterpret of out as (128, 4096)
  y = W_out @ out2 + b_out     (256, 4096)

Sharding: 8 cores = 4 batches x 2 column-chunks (j-axis of the score
matrix = rows of k/v).  Column-softmax stats are local to a j-chunk;
each core produces a partial y, host sums the pair.  The SPMD graph is
identical on all cores; the j-half selection is encoded host-side by
rotating the qkv output channels by 192 for odd cores and rotating
W_out's e-axis by 64 to compensate.

Compute layout (per core), v3:
  head:    x loads + F = W_qkv@x+b (bf16) -> fbuf writes interleaved
           with the qT/kT xbar transposes on the sync ring (the DMA
           device is an exclusive serial resource - issue order is
           priority).  F drains on DVE/ACT(ob0)/Pool(ob1,2) so no
           queue blocks another.  v loads via SWDGE late (tail use).
  phase A: ACT runs ONLY exps (the critical path).  Lead chunks
           (jb0-3 x 1024-wide halves of i<2048) start as soon as
           qT part0/kT part0 land (~10us); then 2048-wide chunks
           (psA = 2 x 4 PSUM banks).  Each exp: 4 score MMs bf16 ->
           PSUM, exp with accum_out -> bf16 ring.
  stats:   DVE: zsum = reduce(zacc); szinv = 1/(zsum/4096) = 4096/Z.
  converts (DVE/Pool/ACT-last, hidden under ACT): dPn8 =
           fp8e4(P*szinv - 1) scattered into P8 so column i'=hb*128+e2
           holds score row i=e2*32+hb (proj2-transpose order).
           v8 = fp8e4(v) plain.
  tail:    colsum[e] = sum_j v_bf16[j,e] (16 trivial MMs; the rank-1
           "1" term of Pn = 1 + dPn); per 512-wide group g: 8 fp8
           DoubleRow pair-MMs (4x PE) accumulate dPn8 @ v8, drain
           with bias=colsum, 4 TensorE transposes, out2 copy, proj2
           (woutT pre-scaled by 1/4096 host-side) + b_out, y -> sync.
"""

import numpy as np
import ml_dtypes

import concourse.bass as bass
import concourse.bacc as bacc
import concourse.mybir as mybir
from concourse.bass_utils import run_bass_kernel_spmd
from concourse.tile import TileContext, add_dep_helper
from concourse.masks import make_identity

BF16 = mybir.dt.bfloat16
F32 = mybir.dt.float32
FP8 = mybir.dt.float8e4
AF = mybir.ActivationFunctionType
ALU = mybir.AluOpType
DR = mybir.MatmulPerfMode.DoubleRow

N, C, E, O, HW = 4, 256, 128, 384, 4096
JC = HW // 2          # j-chunk per core
NJB = JC // 128       # 16 j-blocks
SCALE = 1.0 / 64.0    # 1/sqrt(HW)
SFIX = 4096.0         # softmax renorm: Pn = P * (4096/Z), undone in W_out

_CACHE = {}


def build_nc():
    nc = bacc.Bacc("TRN2", target_bir_lowering=False, debug=False, num_devices=8)

    x_ext = nc.dram_tensor("x", [C, HW], BF16, kind="ExternalInput").ap()
    wqkvT_ext = nc.dram_tensor("wqkvT", [C, O], BF16, kind="ExternalInput").ap()
    bqkv_ext = nc.dram_tensor("bqkv", [O, 1], F32, kind="ExternalInput").ap()
    woutT_ext = nc.dram_tensor("woutT", [E, C], BF16, kind="ExternalInput").ap()
    bout_ext = nc.dram_tensor("bout", [C, 1], F32, kind="ExternalInput").ap()
    y_ext = nc.dram_tensor("out", [C, HW], BF16, kind="ExternalOutput").ap()

    fbuf = nc.dram_tensor("fbuf", [O * HW], BF16).ap()
    fview_o = fbuf.rearrange("(o hw) -> o hw", hw=HW)   # (384, 4096) write view
    fview_i = fbuf.rearrange("(i j) -> i j", j=O)        # (4096, 384) read view

    # persistent SBUF.  qT/kT/v split at 512-aligned boundaries covered by
    # successive F o-tiles.
    QSPL = [0, 1024, 2560, HW]       # parts covered by F o-tiles 0 / 0+1 / 1+2
    KSPL = [0, 1024, JC]
    VSPL = [0, 1280, JC]
    qTp = [nc.alloc_sbuf_tensor(f"qT{i}", [128, QSPL[i + 1] - QSPL[i]], BF16).ap()
           for i in range(3)]
    kTp = [nc.alloc_sbuf_tensor(f"kT{i}", [128, KSPL[i + 1] - KSPL[i]], BF16).ap()
           for i in range(2)]
    vp = [nc.alloc_sbuf_tensor(f"v{i}", [128, VSPL[i + 1] - VSPL[i]], BF16).ap()
          for i in range(2)]

    def qT_sl(i0, w=512):
        p = 0 if i0 < 1024 else (1 if i0 < 2560 else 2)
        a = i0 - QSPL[p]
        assert a + w <= QSPL[p + 1] - QSPL[p]
        return qTp[p][:, a:a + w]

    def kT_sl(jb):
        p = 0 if jb < 8 else 1
        a = jb * 128 - KSPL[p]
        return kTp[p][:, a:a + 128]

    def v_sl(jb):
        p = 0 if jb < 10 else 1
        a = jb * 128 - VSPL[p]
        return vp[p][:, a:a + 128]

    v8 = nc.alloc_sbuf_tensor("v8", [128, JC], FP8).ap()       # (j, e) fp8
    zacc = nc.alloc_sbuf_tensor("zacc", [128, 36], F32).ap()
    zsum = nc.alloc_sbuf_tensor("zsum", [128, 16], F32).ap()
    ztmp = nc.alloc_sbuf_tensor("ztmp", [128, 16], F32).ap()
    szinv = nc.alloc_sbuf_tensor("szinv", [128, 16], F32).ap()  # 4096/Z per jb
    cs_row = nc.alloc_sbuf_tensor("cs_row", [1, 128], BF16).ap()  # colsum(v) row
    out2g = [nc.alloc_sbuf_tensor(f"out2g{g}", [128, 512], BF16).ap()
             for g in range(8)]
    # dPn8: per jb, column i' = hb*128 + e2 holds data for row i = e2*32 + hb.
    P8 = nc.alloc_sbuf_tensor("P8", [128, NJB * HW], FP8).ap()
    # bf16 exp staging: 8 dedicated lead buffers + 6-deep ring for the rest.
    Plead = nc.alloc_sbuf_tensor("Plead", [128, 8 * 1024], BF16).ap()
    PR = 6
    Pring = nc.alloc_sbuf_tensor("Pring", [128, PR * 2048], BF16).ap()

    # zacc slot map: jb<4 -> 3 chunks (A,B,C), jb>=4 -> 2 chunks (H0,H1)
    def zslot(jb, i0):
        if jb < 4:
            return 3 * jb + (0 if i0 == 0 else (1 if i0 == 1024 else 2))
        return 12 + 2 * (jb - 4) + (0 if i0 == 0 else 1)

    def nchunks(jb):
        return 3 if jb < 4 else 2

    def p8_dst(jb, i0, w):
        # out AP dims (e2: w/32 @1 offset i0/32, hb: 32 @128) within jb region
        reg = P8[:, jb * HW:(jb + 1) * HW]
        v = reg.rearrange("p (hb e) -> p e hb", hb=32)
        return v[:, i0 // 32:(i0 + w) // 32, :]

    def p8_lhsT(t, hb):
        # stationary for (pair t, hb-block): dims (p, jb-pair 2 @4096, 128 @1)
        reg = P8[:, t * 2 * HW:(t + 1) * 2 * HW]
        v = reg.rearrange("p (two x) -> p two x", two=2)
        return v[:, :, hb * 128:(hb + 1) * 128]

    with TileContext(nc) as tc:
        with tc.tile_pool(name="consts", bufs=1) as consts:
            # ---- constants: ONE merged DMA for wq (gates stage-1, sync ring
            #      first so it lands before x), rest merged on scalar ----
            wq_all = consts.tile([128, 2 * O], BF16, name="wq_all", tag="wq_all")
            wqT = [wq_all[:, 0:O], wq_all[:, O:2 * O]]
            wq_ld = nc.sync.dma_start(
                out=wq_all[:].rearrange("p (cb o) -> p cb o", cb=2),
                in_=wqkvT_ext.rearrange("(cb p) o -> p cb o", cb=2))
            bias = consts.tile([128, 8], F32, name="bias", tag="bias")
            bq = [bias[:, i:i + 1] for i in range(3)]
            bo = [bias[:, 3 + i:4 + i] for i in range(2)]
            nc.scalar.dma_start(
                out=bias[:, 0:3],
                in_=bqkv_ext.rearrange("(a p) one -> p (a one)", p=128))
            nc.scalar.dma_start(
                out=bias[:, 3:5],
                in_=bout_ext.rearrange("(cb p) one -> p (cb one)", p=128))
            misc = consts.tile([128, C + 8], BF16, name="misc", tag="misc")
            woutT = misc[:, 0:C]
            ones16 = misc[:, C:C + 1]
            nc.scalar.dma_start(out=woutT, in_=woutT_ext[:])
            nc.vector.memset(ones16, 1.0)
            scratch = consts.tile([128, 2], F32, name="scratch", tag="scratch")
            neg1 = scratch[:, 1:2]
            nc.vector.memset(scratch[:, 0:1], 0.0)
            nc.vector.memset(neg1, -1.0)
            nc.scalar.activation(scratch[:, 0:1], scratch[:, 0:1], AF.Exp)

            # ---- PE warmup: dummy matmuls so HAM ramps early ----
            wsrc = consts.tile([128, 128], BF16, name="wsrc", tag="wsrc")
            nc.vector.memset(wsrc[:], 1.0)
            with tc.tile_pool(name="psW", bufs=1, space="PSUM") as psW:
                wtile = psW.tile([128, 128], F32, tag="warm")
                for _ in range(16):
                    nc.tensor.matmul(wtile[:], wsrc[:], wsrc[:], start=True, stop=True)

            # ---- x loads: all on sync ring (SP has nothing else to do) ----
            early = tc.alloc_tile_pool(name="early", bufs=1)
            xsb = [[early.tile([128, HW // 2], BF16, name=f"x{cb}{h}", tag=f"x{cb}{h}")
                    for h in range(2)] for cb in range(2)]
            Fsb = [[early.tile([128, HW // 2], BF16, name=f"F{i}{hh}", tag=f"F{i}{hh}")
                    for hh in range(2)] for i in range(3)]
            xlds = []
            for h in range(2):
                for cb in range(2):
                    r = nc.sync.dma_start(
                        out=xsb[cb][h][:],
                        in_=x_ext[cb * 128:(cb + 1) * 128,
                                  h * (HW // 2):(h + 1) * (HW // 2)])
                    # ring-order hints: wq first, then x in program order
                    add_dep_helper(r.ins, (xlds[-1] if xlds else wq_ld).ins,
                                   sync=False, reason="ring order")
                    xlds.append(r)

            # ---- stage 1: F o-tiles -> fbuf, transposes interleaved on the
            #      sync ring right after the F writes they depend on ----
            # drains: h0 chunks -> DVE; h1 chunks: ob0 -> ACT, ob1/ob2 -> Pool
            # (keeps ACT free for lead exps after ~7us).
            tr_specs = {
                "qT0": (qTp[0], fview_i[0:1024, 0:E], (0,)),
                "kT0": (kTp[0], fview_i[0:1024, E:2 * E], (0,)),
                "qT1": (qTp[1], fview_i[1024:2560, 0:E], (0, 1)),
                "kT1": (kTp[1], fview_i[1024:2048, E:2 * E], (0, 1)),
                "qT2": (qTp[2], fview_i[2560:HW, 0:E], (1, 2)),
            }
            # psL (lead-chunk PSUM) opened BEFORE psF so its banks are
            # disjoint from stage-1's: no anti-deps delaying the lead exps.
            psL = tc.alloc_tile_pool(name="psL", bufs=2, space="PSUM")
            with tc.tile_pool(name="psF", bufs=4, space="PSUM") as psF:
                f_writes = [[] for _ in range(3)]
                ring_last = [xlds[-1]]  # last sync-ring DMA, for order hints

                def launch_tr(key):
                    dst, srcap, deps = tr_specs[key]
                    rt = nc.sync.dma_start_transpose(out=dst[:], in_=srcap)
                    for dep in deps:
                        for w2 in f_writes[dep]:
                            add_dep_helper(rt.ins, w2.ins, reason="fbuf RAW")
                    add_dep_helper(rt.ins, ring_last[0].ins, sync=False,
                                   reason="ring order")
                    ring_last[0] = rt

                for ob in range(3):
                    for nch in range(8):
                        pf = psF.tile([128, 512], F32, tag="pf")
                        h, o512 = nch // 4, (nch % 4) * 512
                        nc.tensor.matmul(
                            pf[:], wqT[0][:, ob * 128:(ob + 1) * 128],
                            xsb[0][h][:, o512:o512 + 512],
                            start=True, stop=False,
                        )
                        nc.tensor.matmul(
                            pf[:], wqT[1][:, ob * 128:(ob + 1) * 128],
                            xsb[1][h][:, o512:o512 + 512],
                            start=False, stop=True,
                        )
                        fb = Fsb[ob][nch // 4]
                        fsl = slice((nch % 4) * 512, (nch % 4 + 1) * 512)
                        if nch < 4:
                            nc.vector.tensor_scalar_add(fb[:, fsl], pf[:], bq[ob])
                        elif ob == 0:
                            nc.scalar.activation(fb[:, fsl], pf[:], AF.Identity,
                                                 bias=bq[ob])
                        else:
                            nc.gpsimd.tensor_scalar_add(fb[:, fsl], pf[:], bq[ob])
                        if nch == 3 or nch == 7:
                            hh = nch // 4
                            fw = nc.sync.dma_start(
                                out=fview_o[ob * 128:(ob + 1) * 128,
                                            hh * (HW // 2):(hh + 1) * (HW // 2)],
                                in_=Fsb[ob][hh][:],
                            )
                            add_dep_helper(fw.ins, ring_last[0].ins, sync=False,
                                           reason="ring order")
                            ring_last[0] = fw
                            f_writes[ob].append(fw)
                    if ob == 0:
                        launch_tr("qT0")
                        launch_tr("kT0")
                    elif ob == 1:
                        launch_tr("qT1")
                        launch_tr("kT1")
                    else:
                        launch_tr("qT2")
                # v loads (SWDGE) issued last: only needed in the tail, and
                # their gather transfers must not delay the transposes on the
                # serialized DMA device.
                for part, (r0, r1) in enumerate([(0, 1280), (1280, 2048)]):
                    r = nc.gpsimd.dma_start(
                        out=vp[part].rearrange("p (t d) -> p t d", d=128),
                        in_=fview_i[r0:r1, 2 * E:3 * E].rearrange(
                            "(t p) d -> p t d", p=128))
                    for w2 in f_writes[0] + f_writes[1]:
                        add_dep_helper(r.ins, w2.ins, reason="fbuf RAW")
            early.release()

            # v8 = fp8(v) plain converts (DVE), as soon as v parts land
            nc.vector.tensor_copy(v8[:, 0:1280], vp[0][:])
            nc.vector.tensor_copy(v8[:, 1280:JC], vp[1][:])

            # ---- phase A ----
            def score_exp(pool, jb, i0, w, buf):
                pa = pool.tile([128, w], F32, tag="pa")
                for n2 in range(w // 512):
                    nc.tensor.matmul(
                        pa[:, n2 * 512:(n2 + 1) * 512],
                        kT_sl(jb), qT_sl(i0 + n2 * 512),
                        start=True, stop=True,
                    )
                zs = zslot(jb, i0)
                nc.scalar.activation(
                    out=buf, in_=pa[:], func=AF.Exp, scale=SCALE,
                    accum_out=zacc[:, zs:zs + 1],
                )

            def stats(jb):
                k = nchunks(jb)
                s0 = zslot(jb, 0)
                nc.vector.reduce_sum(
                    out=zsum[:, jb:jb + 1], in_=zacc[:, s0:s0 + k],
                    axis=mybir.AxisListType.X,
                )
                nc.vector.tensor_scalar_mul(
                    ztmp[:, jb:jb + 1], zsum[:, jb:jb + 1], 1.0 / SFIX)
                nc.vector.reciprocal(szinv[:, jb:jb + 1], ztmp[:, jb:jb + 1])

            def convert(jb, i0, w, buf, eng):
                src = buf.rearrange("p (e hb) -> p e hb", hb=32)
                dst = p8_dst(jb, i0, w)
                if eng is nc.scalar:
                    nc.scalar.activation(dst, src, AF.Identity,
                                         scale=szinv[:, jb:jb + 1], bias=neg1)
                else:
                    eng.tensor_scalar(
                        out=dst, in0=src,
                        scalar1=szinv[:, jb:jb + 1], scalar2=1.0,
                        op0=ALU.mult, op1=ALU.subtract,
                    )

            def conv_eng(jb):
                # alternate DVE/Pool so neither falls behind the exp stream
                return nc.gpsimd if (jb >= 4 and jb % 2 == 0) else nc.vector

            # lead chunks: jb0-3, i<2048 as 1024-wide halves (only need
            # qT part0/1 + kT part0); dedicated buffers (no ring WAR).
            for li, (jb, i0) in enumerate(
                    [(j, 0) for j in range(4)] + [(j, 1024) for j in range(4)]):
                buf = Plead[:, li * 1024:(li + 1) * 1024]
                score_exp(psL, jb, i0, 1024, buf)
            psL.release()

            # main chunks (2048 wide): jb0-3 C first, then jb4-15 pairs.
            mains = [(jb, 2048) for jb in range(4)]
            for jb in range(4, NJB):
                mains += [(jb, 0), (jb, 2048)]
            with tc.tile_pool(name="psA", bufs=2, space="PSUM") as psA:
                for seq, (jb, i0) in enumerate(mains):
                    buf = Pring[:, (seq % PR) * 2048:(seq % PR + 1) * 2048]
                    score_exp(psA, jb, i0, 2048, buf)
                    if i0 == 2048:       # jb complete -> stats + converts
                        stats(jb)
                        if jb < 4:
                            lA = Plead[:, (jb) * 1024:(jb + 1) * 1024]
                            lB = Plead[:, (4 + jb) * 1024:(5 + jb) * 1024]
                            convert(jb, 0, 1024, lA, nc.vector)
                            convert(jb, 1024, 1024, lB, nc.vector)
                            convert(jb, 2048, 2048, buf, nc.vector)
                        else:
                            h0buf = Pring[:, ((seq - 1) % PR) * 2048:
                                          ((seq - 1) % PR + 1) * 2048]
                            e0 = nc.scalar if jb == 15 else conv_eng(jb)
                            convert(jb, 0, 2048, h0buf, e0)
                            convert(jb, 2048, 2048, buf, conv_eng(jb))

            # ---- tail: out2 produced DIRECTLY by fp8 DoubleRow pair-MMs
            #      (P8 stationary): out2[e2, hb*128+e] = sum_j Pn[j,..]*v8[j,e].
            #      The rank-1 "1" term (colsum row) seeds each accumulation. ----
            with tc.tile_pool(name="psCS", bufs=1, space="PSUM") as psCS, \
                 tc.tile_pool(name="psB", bufs=2, space="PSUM") as psB, \
                 tc.tile_pool(name="psY", bufs=2, space="PSUM") as psY, \
                 tc.tile_pool(name="late", bufs=1) as late:
                # colsum row: cs[0, e] = sum_j v[j, e] in bf16
                cs_ps = psCS.tile([1, 128], F32, tag="cs")
                for jb in range(NJB):
                    nc.tensor.matmul(
                        cs_ps[:], ones16, v_sl(jb),
                        start=(jb == 0), stop=(jb == NJB - 1),
                    )
                nc.vector.tensor_copy(cs_row[:], cs_ps[:])
                ones_row = wsrc[0:1, :]

                yg = [[late.tile([128, 512], BF16, name=f"yb{cb}_{g}",
                                 tag=f"yb{cb}_{g}") for g in range(8)]
                      for cb in range(2)]
                for g in range(8):
                    ob_ps = psB.tile([128, 512], F32, tag="ob_ps")
                    for s in range(4):
                        hb = 4 * g + s
                        sl = ob_ps[:, s * 128:(s + 1) * 128]
                        nc.tensor.matmul(
                            sl, ones_row, cs_row[:],
                            start=True, stop=False,
                        )
                        for t in range(NJB // 2):
                            v8pair = v8[:, t * 256:(t + 1) * 256].rearrange(
                                "p (two e) -> p two e", two=2)
                            nc.tensor.matmul(
                                sl, p8_lhsT(t, hb), v8pair,
                                start=False, stop=(t == NJB // 2 - 1),
                                perf_mode=DR,
                            )
                    if g % 3 == 0:
                        nc.scalar.activation(out2g[g][:], ob_ps[:], AF.Identity)
                    elif g % 3 == 1:
                        nc.vector.tensor_copy(out2g[g][:], ob_ps[:])
                    else:
                        nc.gpsimd.tensor_copy(out2g[g][:], ob_ps[:])
                    for cb in range(2):
                        py = psY.tile([128, 512], F32, tag="py")
                        nc.tensor.matmul(
                            py[:], woutT[:, cb * 128:(cb + 1) * 128], out2g[g][:],
                            start=True, stop=True,
                        )
                        dst = yg[cb][g][:]
                        if cb == 0:
                            nc.scalar.activation(dst, py[:], AF.Identity, bias=bo[cb])
                        else:
                            nc.vector.tensor_scalar_add(dst, py[:], bo[cb])
                        nc.sync.dma_start(
                            out=y_ext[cb * 128:(cb + 1) * 128,
                                      g * 512:(g + 1) * 512],
                            in_=dst)

    nc.compile()
    return nc


def get_nc():
    if "nc" not in _CACHE:
        _CACHE["nc"] = build_nc()
    return _CACHE["nc"]


def make_in_maps(x, W_qkv, b_qkv, W_out, b_out):
    x = np.asarray(x, dtype=np.float32)
    W_qkv = np.asarray(W_qkv, dtype=np.float32)
    b_qkv = np.asarray(b_qkv, dtype=np.float32)
    W_out = np.asarray(W_out, dtype=np.float32) / SFIX   # undo Pn renorm
    b_out = np.asarray(b_out, dtype=np.float32)

    operm = (np.arange(O) + O // 2) % O      # rotate qkv channels by 192
    eperm = (np.arange(E) + E // 2) % E      # rotate e-axis by 64

    halves = []
    for h in range(2):
        if h == 0:
            wq, bqv, wo, bov = W_qkv, b_qkv, W_out, b_out
        else:
            wq = W_qkv[operm]
            bqv = b_qkv[operm]
            wo = W_out[:, eperm]
            bov = np.zeros_like(b_out)
        halves.append({
            "wqkvT": np.ascontiguousarray(wq.T).astype(ml_dtypes.bfloat16),
            "bqkv": np.ascontiguousarray(bqv.reshape(O, 1)),
            "woutT": np.ascontiguousarray(wo.T).astype(ml_dtypes.bfloat16),
            "bout": np.ascontiguousarray(bov.reshape(C, 1)),
        })

    xb = [np.ascontiguousarray(x[n].reshape(C, HW)).astype(ml_dtypes.bfloat16)
          for n in range(N)]
    in_maps = []
    for core in range(8):
        n, h = core // 2, core % 2
        m = {"x": xb[n]}
        m.update(halves[h])
        in_maps.append(m)
    return in_maps


def run(inputs, trace=False, **kw):
    nc = get_nc()
    in_maps = make_in_maps(**inputs)
    res = run_bass_kernel_spmd(nc, in_maps, core_ids=list(range(8)), trace=trace, **kw)
    ys = [np.asarray(res.results[i]["out"], dtype=np.float32) for i in range(8)]
    y = np.stack([ys[2 * n] + ys[2 * n + 1] for n in range(N)])
    return y.reshape(N, C, 64, 64), res


def kernel(**inputs):
    y, _ = run(inputs, trace=False)
    return y
